# revision 28
# baseline (speedup 1.0000x reference)
"""Trainium2 Bass kernel for nn_Attention_90220083019846.

Multi-head attention block: q/k/v = X@W{q,k,v}, scores = q@k^T + cb@k^T
(content bias folded into q), softmax, O = P@v, Z = X + O@Wo^T + b, LayerNorm.

Sharding over 8 NeuronCores: data-parallel over batch (2 groups of 4 cores) x
tensor-parallel over heads (4 heads per core). Output projection partial sums
are combined with a chunked ReduceScatter within each batch group; residual +
LayerNorm run on the scattered shards.

Dataflow is fully "transposed": the host passes X^T, so every matmul contracts
over the partition axis with no on-device transposes. Matmuls run in bf16
(f32 PSUM accumulation); softmax sums come free from the P@v matmul via a
fused ones-column in v (M=65). PSUM is organized as two 4-bank slots: score
tiles, their exp-consumers, the P@v partial accumulators and all projection
accumulations rotate through the same two slots, with cross-group O
accumulation done in SBUF by the vector engine.
"""

import contextlib
import ctypes
import sys
import types

sys.path.insert(0, "/opt/trn_rl_repo")

import numpy as np

# ---------------------------------------------------------------- profile hook
# The agent image's antenv lacks axon_hooks; provide it so that
# run_bass_kernel_spmd(trace=True) / BASS_TRACE=1 can capture NTFF profiles.
def _install_profile_hook():
    if "antenv.axon_hooks" in sys.modules:
        return
    try:
        import antenv
    except ImportError:
        return
    mod = types.ModuleType("antenv.axon_hooks")
    mod._hook = None
    mod.set_axon_ntff_profile_hook = lambda h: setattr(mod, "_hook", h)
    mod.get_axon_ntff_profile_hook = lambda: mod._hook
    sys.modules["antenv.axon_hooks"] = mod
    antenv.axon_hooks = mod
    try:
        lib = ctypes.CDLL("/opt/axon/libaxon_pjrt.so")
        if not hasattr(lib, "axon_start_nrt_profile"):
            return
        lib.axon_start_nrt_profile.argtypes = [
            ctypes.POINTER(ctypes.c_int64),
            ctypes.c_size_t,
        ]
        lib.axon_start_nrt_profile.restype = ctypes.c_int64
        lib.axon_stop_nrt_profile.argtypes = [ctypes.c_char_p]
        lib.axon_stop_nrt_profile.restype = ctypes.c_int64

        @contextlib.contextmanager
        def _hook(output_dir, device_ids):
            import jax

            jax.devices()
            if device_ids:
                ids = (ctypes.c_int64 * len(device_ids))(*device_ids)
                rc = lib.axon_start_nrt_profile(ids, len(device_ids))
            else:
                rc = lib.axon_start_nrt_profile(None, 0)
            if rc != 0:
                raise RuntimeError(f"axon_start_nrt_profile rc={rc}")
            try:
                yield
            finally:
                n = lib.axon_stop_nrt_profile(str(output_dir).encode())
                print(f"profile: {n} file(s) written to {output_dir}", file=sys.stderr)

        mod.set_axon_ntff_profile_hook(_hook)
    except OSError:
        pass


_install_profile_hook()

# ------------------------------------------------------------------- constants
B, L, D, H, HD = 2, 2048, 1024, 16, 64
NCORES = 8
GROUP = 4            # cores per batch group (tensor-parallel over heads)
HL = H // GROUP      # local heads per core
DL = HL * HD         # local head dims per core
QB = 512             # query block (tokens per pipeline chunk)
NQB = L // QB
RG = [[0, 1, 2, 3], [4, 5, 6, 7]]
LN_EPS = 1e-5
RSQRT_MAGIC = 0x5F3759DF

_PROGRAM = None
LAST_RESULT = None


def _build_program():
    import concourse.tile as tile
    from concourse import bacc, mybir

    fr = mybir.dt.float32r
    f32 = mybir.dt.float32
    bf16 = mybir.dt.bfloat16
    i32 = mybir.dt.int32
    Exp = mybir.ActivationFunctionType.Exp
    Alu = mybir.AluOpType

    nc = bacc.Bacc("TRN2", target_bir_lowering=False, debug=False,
                   num_devices=NCORES)

    xt_d = nc.dram_tensor("xt", (D, L), bf16, kind="ExternalInput").ap()
    wq_d = nc.dram_tensor("wq", (D, DL), bf16, kind="ExternalInput").ap()
    wk_d = nc.dram_tensor("wk", (D, DL), bf16, kind="ExternalInput").ap()
    wv_d = nc.dram_tensor("wv", (D, DL), bf16, kind="ExternalInput").ap()
    wot_d = nc.dram_tensor("wot", (DL, D), bf16, kind="ExternalInput").ap()
    cb_d = nc.dram_tensor("cb", (DL, 1), f32, kind="ExternalInput").ap()
    xres_d = nc.dram_tensor("xres", (QB, D), f32, kind="ExternalInput").ap()
    wob_d = nc.dram_tensor("wob", (1, D), f32, kind="ExternalInput").ap()
    lng_d = nc.dram_tensor("lng", (1, D), f32, kind="ExternalInput").ap()
    lnb_d = nc.dram_tensor("lnb", (1, D), f32, kind="ExternalInput").ap()
    ones_d = nc.dram_tensor("ones", (128, 128), fr, kind="ExternalInput").ap()
    vones_d = nc.dram_tensor("vones", (128, (L // 128) * HL), bf16,
                             kind="ExternalInput").ap()
    out_d = nc.dram_tensor("out", (QB, D), f32, kind="ExternalOutput").ap()

    ccin = [nc.dram_tensor(f"ccin{t}", (QB, D), f32, kind="Internal").ap()
            for t in range(NQB)]
    ccout = [nc.dram_tensor(f"ccout{t}", (QB // GROUP, D), f32,
                            kind="Internal").ap()
             for t in range(NQB)]

    with tile.TileContext(nc) as tc, contextlib.ExitStack() as ctx:
        # ---------------- persistent pools
        wp = ctx.enter_context(tc.tile_pool(name="wp", bufs=1))
        kqv = ctx.enter_context(tc.tile_pool(name="kqv", bufs=1))
        cons = ctx.enter_context(tc.tile_pool(name="cons", bufs=1))
        # two 3-bank psum slots for scores/projections + a dedicated P@v
        # accumulator pool (2 banks); the sums-broadcast rotates through psp
        psp = ctx.enter_context(tc.tile_pool(name="psp", bufs=2, space="PSUM"))
        ohp = ctx.enter_context(tc.tile_pool(name="ohp", bufs=1, space="PSUM"))
        rbp = ctx.enter_context(tc.tile_pool(name="rbp", bufs=1, space="PSUM"))

        wq_t = wp.tile([128, 8, DL], bf16)
        wk_t = wp.tile([128, 8, DL], bf16)
        wv_t = wp.tile([128, 8, DL], bf16)
        wot_t = wp.tile([128, HL, D], bf16)
        nc.sync.dma_start(out=wq_t, in_=wq_d.rearrange("(c p) o -> p c o", p=128))
        nc.sync.dma_start(out=wk_t, in_=wk_d.rearrange("(c p) o -> p c o", p=128))
        nc.sync.dma_start(out=wv_t, in_=wv_d.rearrange("(c p) o -> p c o", p=128))
        # rows 64..127 of each wot chunk are zero: they pair with the zero
        # rows of the padded ohn stationary operand (full-square matmuls
        # stream 2x faster than K<128 ones)
        nc.gpsimd.memset(wot_t[64:128, :, :], 0.0)
        nc.sync.dma_start(out=wot_t[0:64, :, :],
                          in_=wot_d.rearrange("(h p) e -> p h e", p=64))

        # k^T with the other head of the pair zeroed (full-square lhsT);
        # q^T keeps both heads (zero weights ignore the other head's rows)
        kt_ev = kqv.tile([128, 2, L], bf16)
        kt_od = kqv.tile([128, 2, L], bf16)
        qt = kqv.tile([128, 2, L], bf16)     # q^T (+cb)
        vaug = kqv.tile([128, L // 128, HL * 128], bf16)  # v | ones | zeros
        ohn = kqv.tile([128, HL, L], bf16)   # normalized Oh^T (rows 64+ zero)
        nc.gpsimd.memset(kt_ev[64:128, :, :], 0.0)
        nc.gpsimd.memset(kt_od[0:64, :, :], 0.0)
        nc.gpsimd.memset(vaug, 0.0)
        nc.gpsimd.memset(ohn[64:128, :, :], 0.0)

        cb_t = cons.tile([128, 2], f32)
        nc.sync.dma_start(out=cb_t, in_=cb_d.rearrange("(m p) x -> p (m x)", p=128))
        # lhsT for the sums broadcast: row 64 ones, all else zero (f32r)
        ones_t = cons.tile([128, 128], fr)
        nc.sync.dma_start(out=ones_t, in_=ones_d)
        wob_t = cons.tile([128, D], f32)
        nc.sync.dma_start(out=wob_t, in_=wob_d.partition_broadcast(128))
        lng_t = cons.tile([128, D], f32)
        nc.sync.dma_start(out=lng_t, in_=lng_d.partition_broadcast(128))
        lnb_t = cons.tile([128, D], f32)
        nc.sync.dma_start(out=lnb_t, in_=lnb_d.partition_broadcast(128))
        magic_t = cons.tile([128, 1], i32)
        nc.vector.memset(magic_t, RSQRT_MAGIC)

        nc.sync.dma_start(
            out=vaug.rearrange("p k (h x) -> p k h x", h=HL)[:, :, :, HD:HD + 1],
            in_=vones_d.rearrange("p (k h x) -> p k h x", k=L // 128, h=HL),
        )

        # ---------------- stage A: projections (needs X^T)
        with tc.tile_pool(name="xtp", bufs=1) as xtp:
            xt = xtp.tile([128, 8, L], bf16)
            for c in range(8):
                nc.sync.dma_start(out=xt[:, c, :], in_=xt_d[128 * c:128 * (c + 1), :])

            # k^T / q^T: out-dims on partitions, tokens free
            for m in range(2):
                for t4 in range(NQB):
                    tsl = slice(QB * t4, QB * (t4 + 1))
                    for w_t, is_q in ((wk_t, False), (wq_t, True)):
                        ps = psp.tile([128, 3 * 512], f32, tag="ps")
                        for c in range(8):
                            nc.tensor.matmul(
                                out=ps[:, 0:512],
                                lhsT=w_t[:, c, 128 * m:128 * (m + 1)],
                                rhs=xt[:, c, tsl],
                                start=(c == 0), stop=(c == 7),
                            )
                        if is_q:
                            nc.vector.tensor_scalar_add(
                                out=qt[:, m, tsl], in0=ps[:, 0:512],
                                scalar1=cb_t[:, m:m + 1])
                        else:
                            nc.scalar.copy(out=kt_ev[0:64, m, tsl],
                                           in_=ps[0:64, 0:512])
                            nc.vector.tensor_copy(out=kt_od[64:128, m, tsl],
                                                  in_=ps[64:128, 0:512])

            # v: tokens on partitions, head dims free (augmented with ones col)
            for kc in range(L // 128):
                ps = psp.tile([128, 3 * 512], f32, tag="ps")
                for c in range(8):
                    nc.tensor.matmul(
                        out=ps[:, 0:DL],
                        lhsT=xt[:, c, 128 * kc:128 * (kc + 1)],
                        rhs=wv_t[:, c, :],
                        start=(c == 0), stop=(c == 7),
                    )
                nc.vector.tensor_copy(
                    out=vaug[:, kc, :].rearrange("p (h x) -> p h x", h=HL)[:, :, 0:HD],
                    in_=ps[:, 0:DL].rearrange("p (h x) -> p h x", h=HL),
                )
                # vaug columns 64 (ones) and 65..127 (zeros) were set up front

        # ---------------- stage B (attention) + stage C (proj/RS/LN)
        with tc.tile_pool(name="ptp", bufs=3) as ptp, \
             tc.tile_pool(name="ohsp", bufs=2) as ohsp, \
             tc.tile_pool(name="recp", bufs=2) as recp, \
             tc.tile_pool(name="zevp", bufs=2) as zevp, \
             tc.tile_pool(name="lnp", bufs=2) as lnp:

            groups = [(0, 3), (3, 3), (6, 3), (9, 3), (12, 3), (15, 1)]
            nkc = L // 128

            for qb in range(NQB):
                # ---- attention for all local heads on this query block
                for h in range(HL):
                    mi = h // 2
                    ktp = kt_ev if h % 2 == 0 else kt_od
                    qT_b = qt[:, mi, QB * qb:QB * (qb + 1)]
                    oh = ohp.tile([128, 512], f32, tag="oh")
                    for kc0, n in groups:
                        st = psp.tile([128, 3 * 512], f32, tag="ps")
                        for i in range(n):
                            kc = kc0 + i
                            nc.tensor.matmul(
                                out=st[:, 512 * i:512 * (i + 1)],
                                lhsT=ktp[:, mi, 128 * kc:128 * (kc + 1)],
                                rhs=qT_b,
                                start=True, stop=True,
                            )
                        pt = ptp.tile([128, 3 * 512], bf16, tag="pt")
                        nc.scalar.activation(out=pt[:, :512 * n],
                                             in_=st[:, :512 * n], func=Exp)
                        for i in range(n):
                            kc = kc0 + i
                            nc.tensor.matmul(
                                out=oh,
                                lhsT=vaug[:, kc, 128 * h:128 * (h + 1)],
                                rhs=pt[:, 512 * i:512 * (i + 1)],
                                start=(kc == 0), stop=(kc == nkc - 1),
                            )
                    # evacuate Oh + sums, broadcast sums, reciprocal, scale
                    ohs = ohsp.tile([65, 512], fr, tag="ohs")
                    with nc.allow_low_precision(reason="f32r rounding of Oh"):
                        nc.vector.tensor_copy(out=ohs, in_=oh[0:65, :])
                    rb = rbp.tile([128, 512], f32, tag="rb")
                    nc.tensor.matmul(out=rb, lhsT=ones_t[0:65, :],
                                     rhs=ohs, start=True, stop=True)
                    rbs = recp.tile([64, 512], f32, tag="rbs")
                    nc.vector.tensor_copy(out=rbs, in_=rb[0:64, :])
                    rec = recp.tile([64, 512], f32, tag="rec")
                    nc.vector.reciprocal(out=rec, in_=rbs)
                    nc.vector.tensor_mul(
                        out=ohn[0:64, h, QB * qb:QB * (qb + 1)],
                        in0=ohs[0:64, :], in1=rec)

                # ---- output projection partial for this token block
                for tcl in range(QB // 128):
                    t0 = QB * qb + 128 * tcl
                    zev = zevp.tile([128, D], f32)
                    for ec in range(2):
                        zp = psp.tile([128, 3 * 512], f32, tag="ps")
                        for h in range(HL):
                            nc.tensor.matmul(
                                out=zp[:, 0:512],
                                lhsT=ohn[:, h, t0:t0 + 128],
                                rhs=wot_t[:, h, 512 * ec:512 * (ec + 1)],
                                start=(h == 0), stop=(h == HL - 1),
                            )
                        nc.vector.tensor_copy(out=zev[:, 512 * ec:512 * (ec + 1)],
                                              in_=zp[:, 0:512])
                    nc.sync.dma_start(out=ccin[qb][128 * tcl:128 * (tcl + 1), :],
                                      in_=zev)

                # ---- combine partials across the batch group
                nc.gpsimd.collective_compute(
                    "ReduceScatter", Alu.add,
                    ins=[ccin[qb][:]], outs=[ccout[qb][:]],
                    replica_groups=RG,
                )

            # ---- residual + bias + LayerNorm, deferred so the in-order DVE
            # stream never blocks attention work behind a ReduceScatter wait
            for qb in range(NQB):
                zt = lnp.tile([128, D], f32, tag="zt")
                nc.sync.dma_start(out=zt, in_=ccout[qb])
                xr = lnp.tile([128, D], f32, tag="xr")
                nc.sync.dma_start(out=xr, in_=xres_d[128 * qb:128 * (qb + 1), :])
                nc.vector.tensor_add(out=zt, in0=zt, in1=xr)
                nc.vector.tensor_add(out=zt, in0=zt, in1=wob_t)

                stats = lnp.tile([128, 2, 6], f32, tag="stats")
                for sg in range(2):
                    nc.vector.bn_stats(out=stats[:, sg, :],
                                       in_=zt[:, 512 * sg:512 * (sg + 1)])
                mv = lnp.tile([128, 2], f32, tag="mv")
                nc.vector.bn_aggr(out=mv, in_=stats)

                # rstd = rsqrt(var + eps), DVE-only (avoids ACT table thrash)
                ve = lnp.tile([128, 1], f32, tag="ve")
                nc.vector.tensor_scalar_add(out=ve, in0=mv[:, 1:2], scalar1=LN_EPS)
                y = lnp.tile([128, 1], f32, tag="y")
                nc.vector.tensor_scalar(
                    out=y.bitcast(i32), in0=ve.bitcast(i32), scalar1=1,
                    scalar2=None, op0=Alu.logical_shift_right)
                nc.vector.tensor_sub(out=y.bitcast(i32), in0=magic_t,
                                     in1=y.bitcast(i32))
                tnw = lnp.tile([128, 1], f32, tag="tnw")
                for _ in range(3):
                    nc.vector.tensor_mul(out=tnw, in0=ve, in1=y)
                    nc.vector.tensor_mul(out=tnw, in0=tnw, in1=y)
                    nc.vector.tensor_scalar(out=tnw, in0=tnw, scalar1=-0.5,
                                            scalar2=1.5, op0=Alu.mult, op1=Alu.add)
                    nc.vector.tensor_mul(out=y, in0=y, in1=tnw)

                nc.vector.tensor_scalar(out=zt, in0=zt, scalar1=mv[:, 0:1],
                                        scalar2=y, op0=Alu.subtract, op1=Alu.mult)
                nc.vector.tensor_mul(out=zt, in0=zt, in1=lng_t)
                nc.vector.tensor_add(out=zt, in0=zt, in1=lnb_t)
                nc.sync.dma_start(out=out_d[128 * qb:128 * (qb + 1), :], in_=zt)


    nc.compile()
    return nc


def _get_program():
    global _PROGRAM
    if _PROGRAM is None:
        _PROGRAM = _build_program()
    return _PROGRAM


def kernel(X, Y, Wq, Wk, Wv, cb, Wo_w, Wo_b, ln_g, ln_b):
    import ml_dtypes
    from concourse import bass_utils

    prog = _get_program()
    bf = ml_dtypes.bfloat16

    X = np.asarray(X, dtype=np.float32)
    Wq = np.asarray(Wq, dtype=np.float32)
    Wk = np.asarray(Wk, dtype=np.float32)
    Wv = np.asarray(Wv, dtype=np.float32)
    cb = np.asarray(cb, dtype=np.float32)
    Wo_w = np.asarray(Wo_w, dtype=np.float32)
    Wo_b = np.asarray(Wo_b, dtype=np.float32)
    ln_g = np.asarray(ln_g, dtype=np.float32)
    ln_b = np.asarray(ln_b, dtype=np.float32)

    WoT = np.ascontiguousarray(Wo_w.T)
    ones_arr = np.zeros((128, 128), np.float32)
    ones_arr[64, :] = 1.0
    in_maps = []
    for c in range(NCORES):
        b, hp, r = c // GROUP, c % GROUP, c % GROUP
        Xb = X[b]
        rows = np.concatenate(
            [np.arange(QB * t + 128 * r, QB * t + 128 * r + 128)
             for t in range(NQB)])
        csl = slice(DL * hp, DL * (hp + 1))
        in_maps.append({
            "xt": np.ascontiguousarray(Xb.T).astype(bf),
            "xres": np.ascontiguousarray(Xb[rows]),
            "wq": np.ascontiguousarray(Wq[:, csl]).astype(bf),
            "wk": np.ascontiguousarray(Wk[:, csl]).astype(bf),
            "wv": np.ascontiguousarray(Wv[:, csl]).astype(bf),
            "wot": np.ascontiguousarray(WoT[csl, :]).astype(bf),
            "cb": np.ascontiguousarray(cb[csl].reshape(DL, 1)),
            "wob": np.ascontiguousarray(Wo_b.reshape(1, D)),
            "lng": np.ascontiguousarray(ln_g.reshape(1, D)),
            "lnb": np.ascontiguousarray(ln_b.reshape(1, D)),
            "ones": ones_arr,
            "vones": np.ones((128, (L // 128) * HL), bf),
        })

    res = bass_utils.run_bass_kernel_spmd(prog, in_maps, core_ids=list(range(NCORES)))
    global LAST_RESULT
    LAST_RESULT = res

    out = np.empty((B, L, D), np.float32)
    for c in range(NCORES):
        b, r = c // GROUP, c % GROUP
        o = res.results[c]["out"]
        for t in range(NQB):
            out[b, QB * t + 128 * r:QB * t + 128 * r + 128] = o[128 * t:128 * (t + 1)]
    return out


if __name__ == "__main__":
    rng = np.random.default_rng(0)
    ins = {
        "X": rng.standard_normal((B, L, D)).astype(np.float32),
        "Y": rng.standard_normal((B, L, D)).astype(np.float32),
        "Wq": (rng.uniform(-1, 1, (D, D)) / 32).astype(np.float32),
        "Wk": (rng.uniform(-1, 1, (D, D)) / 32).astype(np.float32),
        "Wv": (rng.uniform(-1, 1, (D, D)) / 32).astype(np.float32),
        "cb": np.zeros(D, np.float32),
        "Wo_w": (rng.uniform(-1, 1, (D, D)) / 32).astype(np.float32),
        "Wo_b": (rng.uniform(-1, 1, D) / 32).astype(np.float32),
        "ln_g": np.ones(D, np.float32),
        "ln_b": np.zeros(D, np.float32),
    }
    out = kernel(**ins)
    print("out", out.shape, out.dtype, float(np.abs(out).max()))
    print("exec_time_ns:", LAST_RESULT.exec_time_ns)


# revision 29
# speedup vs baseline: 1.0592x; 1.0592x over previous
"""Trainium2 Bass kernel for nn_Attention_90220083019846.

Multi-head attention block: q/k/v = X@W{q,k,v}, scores = q@k^T + cb@k^T
(content bias folded into q), softmax, O = P@v, Z = X + O@Wo^T + b, LayerNorm.

Sharding over 8 NeuronCores: data-parallel over batch (2 groups of 4 cores) x
tensor-parallel over heads (4 heads per core). Output projection partial sums
are combined with a chunked ReduceScatter within each batch group; residual +
LayerNorm run on the scattered shards.

Dataflow is fully "transposed": the host passes X^T, so every matmul contracts
over the partition axis with no on-device transposes. Matmuls run in bf16
(f32 PSUM accumulation); softmax sums come free from the P@v matmul via a
fused ones-column in v (M=65). PSUM is organized as two 4-bank slots: score
tiles, their exp-consumers, the P@v partial accumulators and all projection
accumulations rotate through the same two slots, with cross-group O
accumulation done in SBUF by the vector engine.
"""

import contextlib
import ctypes
import sys
import types

sys.path.insert(0, "/opt/trn_rl_repo")

import numpy as np

# ---------------------------------------------------------------- profile hook
# The agent image's antenv lacks axon_hooks; provide it so that
# run_bass_kernel_spmd(trace=True) / BASS_TRACE=1 can capture NTFF profiles.
def _install_profile_hook():
    if "antenv.axon_hooks" in sys.modules:
        return
    try:
        import antenv
    except ImportError:
        return
    mod = types.ModuleType("antenv.axon_hooks")
    mod._hook = None
    mod.set_axon_ntff_profile_hook = lambda h: setattr(mod, "_hook", h)
    mod.get_axon_ntff_profile_hook = lambda: mod._hook
    sys.modules["antenv.axon_hooks"] = mod
    antenv.axon_hooks = mod
    try:
        lib = ctypes.CDLL("/opt/axon/libaxon_pjrt.so")
        if not hasattr(lib, "axon_start_nrt_profile"):
            return
        lib.axon_start_nrt_profile.argtypes = [
            ctypes.POINTER(ctypes.c_int64),
            ctypes.c_size_t,
        ]
        lib.axon_start_nrt_profile.restype = ctypes.c_int64
        lib.axon_stop_nrt_profile.argtypes = [ctypes.c_char_p]
        lib.axon_stop_nrt_profile.restype = ctypes.c_int64

        @contextlib.contextmanager
        def _hook(output_dir, device_ids):
            import jax

            jax.devices()
            if device_ids:
                ids = (ctypes.c_int64 * len(device_ids))(*device_ids)
                rc = lib.axon_start_nrt_profile(ids, len(device_ids))
            else:
                rc = lib.axon_start_nrt_profile(None, 0)
            if rc != 0:
                raise RuntimeError(f"axon_start_nrt_profile rc={rc}")
            try:
                yield
            finally:
                n = lib.axon_stop_nrt_profile(str(output_dir).encode())
                print(f"profile: {n} file(s) written to {output_dir}", file=sys.stderr)

        mod.set_axon_ntff_profile_hook(_hook)
    except OSError:
        pass


_install_profile_hook()

# ------------------------------------------------------------------- constants
B, L, D, H, HD = 2, 2048, 1024, 16, 64
NCORES = 8
GROUP = 4            # cores per batch group (tensor-parallel over heads)
HL = H // GROUP      # local heads per core
DL = HL * HD         # local head dims per core
QB = 512             # query block (tokens per pipeline chunk)
NQB = L // QB
RG = [[0, 1, 2, 3], [4, 5, 6, 7]]
LN_EPS = 1e-5
RSQRT_MAGIC = 0x5F3759DF

_PROGRAM = None
LAST_RESULT = None


def _build_program():
    import concourse.tile as tile
    from concourse import bacc, mybir

    fr = mybir.dt.float32r
    f32 = mybir.dt.float32
    bf16 = mybir.dt.bfloat16
    i32 = mybir.dt.int32
    Exp = mybir.ActivationFunctionType.Exp
    Alu = mybir.AluOpType

    nc = bacc.Bacc("TRN2", target_bir_lowering=False, debug=False,
                   num_devices=NCORES)

    xt_d = nc.dram_tensor("xt", (D, L), bf16, kind="ExternalInput").ap()
    wq_d = nc.dram_tensor("wq", (D, DL), bf16, kind="ExternalInput").ap()
    wk_d = nc.dram_tensor("wk", (D, DL), bf16, kind="ExternalInput").ap()
    wv_d = nc.dram_tensor("wv", (D, DL), bf16, kind="ExternalInput").ap()
    wot_d = nc.dram_tensor("wot", (DL, D), bf16, kind="ExternalInput").ap()
    cb_d = nc.dram_tensor("cb", (DL, 1), f32, kind="ExternalInput").ap()
    xres_d = nc.dram_tensor("xres", (QB, D), f32, kind="ExternalInput").ap()
    wob_d = nc.dram_tensor("wob", (1, D), f32, kind="ExternalInput").ap()
    lng_d = nc.dram_tensor("lng", (1, D), f32, kind="ExternalInput").ap()
    lnb_d = nc.dram_tensor("lnb", (1, D), f32, kind="ExternalInput").ap()
    ones_d = nc.dram_tensor("ones", (128, 128), fr, kind="ExternalInput").ap()
    vones_d = nc.dram_tensor("vones", (128, (L // 128) * HL), bf16,
                             kind="ExternalInput").ap()
    out_d = nc.dram_tensor("out", (QB, D), f32, kind="ExternalOutput").ap()

    ccin = [nc.dram_tensor(f"ccin{t}", (QB, D), f32, kind="Internal").ap()
            for t in range(NQB)]
    ccout = [nc.dram_tensor(f"ccout{t}", (QB // GROUP, D), f32,
                            kind="Internal").ap()
             for t in range(NQB)]

    with tile.TileContext(nc) as tc, contextlib.ExitStack() as ctx:
        # ---------------- persistent pools
        wp = ctx.enter_context(tc.tile_pool(name="wp", bufs=1))
        kqv = ctx.enter_context(tc.tile_pool(name="kqv", bufs=1))
        cons = ctx.enter_context(tc.tile_pool(name="cons", bufs=1))
        # two 3-bank psum slots for scores/projections + a dedicated P@v
        # accumulator pool (2 banks); the sums-broadcast rotates through psp
        psp = ctx.enter_context(tc.tile_pool(name="psp", bufs=2, space="PSUM"))
        ohp = ctx.enter_context(tc.tile_pool(name="ohp", bufs=1, space="PSUM"))
        rbp = ctx.enter_context(tc.tile_pool(name="rbp", bufs=1, space="PSUM"))

        wq_t = wp.tile([128, 8, DL], bf16)
        wk_t = wp.tile([128, 8, DL], bf16)
        wv_t = wp.tile([128, 8, DL], bf16)
        wot_t = wp.tile([128, HL, D], bf16)
        nc.sync.dma_start(out=wq_t, in_=wq_d.rearrange("(c p) o -> p c o", p=128))
        nc.sync.dma_start(out=wk_t, in_=wk_d.rearrange("(c p) o -> p c o", p=128))
        nc.sync.dma_start(out=wv_t, in_=wv_d.rearrange("(c p) o -> p c o", p=128))
        # rows 64..127 of each wot chunk are zero: they pair with the zero
        # rows of the padded ohn stationary operand (full-square matmuls
        # stream 2x faster than K<128 ones)
        nc.gpsimd.memset(wot_t[64:128, :, :], 0.0)
        nc.sync.dma_start(out=wot_t[0:64, :, :],
                          in_=wot_d.rearrange("(h p) e -> p h e", p=64))

        # k^T with the other head of the pair zeroed (full-square lhsT);
        # q^T keeps both heads (zero weights ignore the other head's rows)
        kt_ev = kqv.tile([128, 2, L], bf16)
        kt_od = kqv.tile([128, 2, L], bf16)
        qt = kqv.tile([128, 2, L], bf16)     # q^T (+cb)
        vaug = kqv.tile([128, L // 128, HL * 128], bf16)  # v | ones | zeros
        ohn = kqv.tile([128, HL, L], bf16)   # normalized Oh^T (rows 64+ zero)
        nc.gpsimd.memset(kt_ev[64:128, :, :], 0.0)
        nc.gpsimd.memset(kt_od[0:64, :, :], 0.0)
        nc.gpsimd.memset(vaug, 0.0)
        nc.gpsimd.memset(ohn[64:128, :, :], 0.0)

        cb_t = cons.tile([128, 2], f32)
        nc.sync.dma_start(out=cb_t, in_=cb_d.rearrange("(m p) x -> p (m x)", p=128))
        # lhsT for the sums broadcast: row 64 ones, all else zero (f32r)
        ones_t = cons.tile([128, 128], fr)
        nc.sync.dma_start(out=ones_t, in_=ones_d)
        wob_t = cons.tile([128, D], f32)
        nc.sync.dma_start(out=wob_t, in_=wob_d.partition_broadcast(128))
        lng_t = cons.tile([128, D], f32)
        nc.sync.dma_start(out=lng_t, in_=lng_d.partition_broadcast(128))
        lnb_t = cons.tile([128, D], f32)
        nc.sync.dma_start(out=lnb_t, in_=lnb_d.partition_broadcast(128))
        magic_t = cons.tile([128, 1], i32)
        nc.vector.memset(magic_t, RSQRT_MAGIC)

        nc.sync.dma_start(
            out=vaug.rearrange("p k (h x) -> p k h x", h=HL)[:, :, :, HD:HD + 1],
            in_=vones_d.rearrange("p (k h x) -> p k h x", k=L // 128, h=HL),
        )

        # ---------------- stage A: projections (needs X^T)
        with tc.tile_pool(name="xtp", bufs=1) as xtp:
            xt = xtp.tile([128, 8, L], bf16)
            for c in range(8):
                nc.sync.dma_start(out=xt[:, c, :], in_=xt_d[128 * c:128 * (c + 1), :])

            # k^T / q^T: out-dims on partitions, tokens free
            for m in range(2):
                for t4 in range(NQB):
                    tsl = slice(QB * t4, QB * (t4 + 1))
                    for w_t, is_q in ((wk_t, False), (wq_t, True)):
                        ps = psp.tile([128, 3 * 512], f32, tag="ps")
                        for c in range(8):
                            nc.tensor.matmul(
                                out=ps[:, 0:512],
                                lhsT=w_t[:, c, 128 * m:128 * (m + 1)],
                                rhs=xt[:, c, tsl],
                                start=(c == 0), stop=(c == 7),
                            )
                        if is_q:
                            nc.vector.tensor_scalar_add(
                                out=qt[:, m, tsl], in0=ps[:, 0:512],
                                scalar1=cb_t[:, m:m + 1])
                        else:
                            nc.scalar.copy(out=kt_ev[0:64, m, tsl],
                                           in_=ps[0:64, 0:512])
                            nc.vector.tensor_copy(out=kt_od[64:128, m, tsl],
                                                  in_=ps[64:128, 0:512])

            # v: tokens on partitions, head dims free (augmented with ones col)
            for kc in range(L // 128):
                ps = psp.tile([128, 3 * 512], f32, tag="ps")
                for c in range(8):
                    nc.tensor.matmul(
                        out=ps[:, 0:DL],
                        lhsT=xt[:, c, 128 * kc:128 * (kc + 1)],
                        rhs=wv_t[:, c, :],
                        start=(c == 0), stop=(c == 7),
                    )
                nc.vector.tensor_copy(
                    out=vaug[:, kc, :].rearrange("p (h x) -> p h x", h=HL)[:, :, 0:HD],
                    in_=ps[:, 0:DL].rearrange("p (h x) -> p h x", h=HL),
                )
                # vaug columns 64 (ones) and 65..127 (zeros) were set up front

        # ---------------- stage B (attention) + stage C (proj/RS/LN)
        with tc.tile_pool(name="ptp", bufs=3) as ptp, \
             tc.tile_pool(name="ohsp", bufs=2) as ohsp, \
             tc.tile_pool(name="recp", bufs=2) as recp, \
             tc.tile_pool(name="zevp", bufs=2) as zevp, \
             tc.tile_pool(name="lnp", bufs=2) as lnp:

            groups = [(0, 3), (3, 3), (6, 3), (9, 3), (12, 3), (15, 1)]
            nkc = L // 128

            for qb in range(NQB):
                # ---- attention for all local heads on this query block
                for h in range(HL):
                    mi = h // 2
                    ktp = kt_ev if h % 2 == 0 else kt_od
                    qT_b = qt[:, mi, QB * qb:QB * (qb + 1)]
                    oh = ohp.tile([128, 512], f32, tag="oh")
                    for kc0, n in groups:
                        st = psp.tile([128, 3 * 512], f32, tag="ps")
                        for i in range(n):
                            kc = kc0 + i
                            nc.tensor.matmul(
                                out=st[:, 512 * i:512 * (i + 1)],
                                lhsT=ktp[:, mi, 128 * kc:128 * (kc + 1)],
                                rhs=qT_b,
                                start=True, stop=True,
                            )
                        pt = ptp.tile([128, 3 * 512], bf16, tag="pt")
                        nc.scalar.activation(out=pt[:, :512 * n],
                                             in_=st[:, :512 * n], func=Exp)
                        for i in range(n):
                            kc = kc0 + i
                            nc.tensor.matmul(
                                out=oh,
                                lhsT=vaug[:, kc, 128 * h:128 * (h + 1)],
                                rhs=pt[:, 512 * i:512 * (i + 1)],
                                start=(kc == 0), stop=(kc == nkc - 1),
                            )
                    # evacuate Oh + sums, broadcast sums, reciprocal, scale
                    ohs = ohsp.tile([65, 512], fr, tag="ohs")
                    with nc.allow_low_precision(reason="f32r rounding of Oh"):
                        nc.vector.tensor_copy(out=ohs, in_=oh[0:65, :])
                    rb = rbp.tile([128, 512], f32, tag="rb")
                    nc.tensor.matmul(out=rb, lhsT=ones_t[0:65, :],
                                     rhs=ohs, start=True, stop=True)
                    rbs = recp.tile([64, 512], f32, tag="rbs")
                    nc.vector.tensor_copy(out=rbs, in_=rb[0:64, :])
                    rec = recp.tile([64, 512], f32, tag="rec")
                    nc.vector.reciprocal(out=rec, in_=rbs)
                    nc.vector.tensor_mul(
                        out=ohn[0:64, h, QB * qb:QB * (qb + 1)],
                        in0=ohs[0:64, :], in1=rec)

                # ---- output projection partial for this token block
                for tcl in range(QB // 128):
                    t0 = QB * qb + 128 * tcl
                    zev = zevp.tile([128, D], f32)
                    for ec in range(2):
                        zp = psp.tile([128, 3 * 512], f32, tag="ps")
                        for h in range(HL):
                            nc.tensor.matmul(
                                out=zp[:, 0:512],
                                lhsT=ohn[:, h, t0:t0 + 128],
                                rhs=wot_t[:, h, 512 * ec:512 * (ec + 1)],
                                start=(h == 0), stop=(h == HL - 1),
                            )
                        nc.vector.tensor_copy(out=zev[:, 512 * ec:512 * (ec + 1)],
                                              in_=zp[:, 0:512])
                    nc.sync.dma_start(out=ccin[qb][128 * tcl:128 * (tcl + 1), :],
                                      in_=zev)

                # ---- combine partials across the batch group
                nc.gpsimd.collective_compute(
                    "ReduceScatter", Alu.add,
                    ins=[ccin[qb][:]], outs=[ccout[qb][:]],
                    replica_groups=RG,
                )

            # ---- residual + bias + LayerNorm, deferred so the in-order DVE
            # stream never blocks attention work behind a ReduceScatter wait
            for qb in range(NQB):
              with tc.tile_wait_until(0.30 + 0.03 * qb):
                zt = lnp.tile([128, D], f32, tag="zt")
                nc.sync.dma_start(out=zt, in_=ccout[qb])
                xr = lnp.tile([128, D], f32, tag="xr")
                nc.sync.dma_start(out=xr, in_=xres_d[128 * qb:128 * (qb + 1), :])
                nc.vector.tensor_add(out=zt, in0=zt, in1=xr)
                nc.vector.tensor_add(out=zt, in0=zt, in1=wob_t)

                stats = lnp.tile([128, 2, 6], f32, tag="stats")
                for sg in range(2):
                    nc.vector.bn_stats(out=stats[:, sg, :],
                                       in_=zt[:, 512 * sg:512 * (sg + 1)])
                mv = lnp.tile([128, 2], f32, tag="mv")
                nc.vector.bn_aggr(out=mv, in_=stats)

                # rstd = rsqrt(var + eps), DVE-only (avoids ACT table thrash)
                ve = lnp.tile([128, 1], f32, tag="ve")
                nc.vector.tensor_scalar_add(out=ve, in0=mv[:, 1:2], scalar1=LN_EPS)
                y = lnp.tile([128, 1], f32, tag="y")
                nc.vector.tensor_scalar(
                    out=y.bitcast(i32), in0=ve.bitcast(i32), scalar1=1,
                    scalar2=None, op0=Alu.logical_shift_right)
                nc.vector.tensor_sub(out=y.bitcast(i32), in0=magic_t,
                                     in1=y.bitcast(i32))
                tnw = lnp.tile([128, 1], f32, tag="tnw")
                for _ in range(3):
                    nc.vector.tensor_mul(out=tnw, in0=ve, in1=y)
                    nc.vector.tensor_mul(out=tnw, in0=tnw, in1=y)
                    nc.vector.tensor_scalar(out=tnw, in0=tnw, scalar1=-0.5,
                                            scalar2=1.5, op0=Alu.mult, op1=Alu.add)
                    nc.vector.tensor_mul(out=y, in0=y, in1=tnw)

                nc.vector.tensor_scalar(out=zt, in0=zt, scalar1=mv[:, 0:1],
                                        scalar2=y, op0=Alu.subtract, op1=Alu.mult)
                nc.vector.tensor_mul(out=zt, in0=zt, in1=lng_t)
                nc.vector.tensor_add(out=zt, in0=zt, in1=lnb_t)
                nc.sync.dma_start(out=out_d[128 * qb:128 * (qb + 1), :], in_=zt)


    nc.compile()
    return nc


def _get_program():
    global _PROGRAM
    if _PROGRAM is None:
        _PROGRAM = _build_program()
    return _PROGRAM


def kernel(X, Y, Wq, Wk, Wv, cb, Wo_w, Wo_b, ln_g, ln_b):
    import ml_dtypes
    from concourse import bass_utils

    prog = _get_program()
    bf = ml_dtypes.bfloat16

    X = np.asarray(X, dtype=np.float32)
    Wq = np.asarray(Wq, dtype=np.float32)
    Wk = np.asarray(Wk, dtype=np.float32)
    Wv = np.asarray(Wv, dtype=np.float32)
    cb = np.asarray(cb, dtype=np.float32)
    Wo_w = np.asarray(Wo_w, dtype=np.float32)
    Wo_b = np.asarray(Wo_b, dtype=np.float32)
    ln_g = np.asarray(ln_g, dtype=np.float32)
    ln_b = np.asarray(ln_b, dtype=np.float32)

    WoT = np.ascontiguousarray(Wo_w.T)
    ones_arr = np.zeros((128, 128), np.float32)
    ones_arr[64, :] = 1.0
    in_maps = []
    for c in range(NCORES):
        b, hp, r = c // GROUP, c % GROUP, c % GROUP
        Xb = X[b]
        rows = np.concatenate(
            [np.arange(QB * t + 128 * r, QB * t + 128 * r + 128)
             for t in range(NQB)])
        csl = slice(DL * hp, DL * (hp + 1))
        in_maps.append({
            "xt": np.ascontiguousarray(Xb.T).astype(bf),
            "xres": np.ascontiguousarray(Xb[rows]),
            "wq": np.ascontiguousarray(Wq[:, csl]).astype(bf),
            "wk": np.ascontiguousarray(Wk[:, csl]).astype(bf),
            "wv": np.ascontiguousarray(Wv[:, csl]).astype(bf),
            "wot": np.ascontiguousarray(WoT[csl, :]).astype(bf),
            "cb": np.ascontiguousarray(cb[csl].reshape(DL, 1)),
            "wob": np.ascontiguousarray(Wo_b.reshape(1, D)),
            "lng": np.ascontiguousarray(ln_g.reshape(1, D)),
            "lnb": np.ascontiguousarray(ln_b.reshape(1, D)),
            "ones": ones_arr,
            "vones": np.ones((128, (L // 128) * HL), bf),
        })

    res = bass_utils.run_bass_kernel_spmd(prog, in_maps, core_ids=list(range(NCORES)))
    global LAST_RESULT
    LAST_RESULT = res

    out = np.empty((B, L, D), np.float32)
    for c in range(NCORES):
        b, r = c // GROUP, c % GROUP
        o = res.results[c]["out"]
        for t in range(NQB):
            out[b, QB * t + 128 * r:QB * t + 128 * r + 128] = o[128 * t:128 * (t + 1)]
    return out


if __name__ == "__main__":
    rng = np.random.default_rng(0)
    ins = {
        "X": rng.standard_normal((B, L, D)).astype(np.float32),
        "Y": rng.standard_normal((B, L, D)).astype(np.float32),
        "Wq": (rng.uniform(-1, 1, (D, D)) / 32).astype(np.float32),
        "Wk": (rng.uniform(-1, 1, (D, D)) / 32).astype(np.float32),
        "Wv": (rng.uniform(-1, 1, (D, D)) / 32).astype(np.float32),
        "cb": np.zeros(D, np.float32),
        "Wo_w": (rng.uniform(-1, 1, (D, D)) / 32).astype(np.float32),
        "Wo_b": (rng.uniform(-1, 1, D) / 32).astype(np.float32),
        "ln_g": np.ones(D, np.float32),
        "ln_b": np.zeros(D, np.float32),
    }
    out = kernel(**ins)
    print("out", out.shape, out.dtype, float(np.abs(out).max()))
    print("exec_time_ns:", LAST_RESULT.exec_time_ns)


# revision 32
# speedup vs baseline: 1.0989x; 1.0375x over previous
"""Trainium2 Bass kernel for nn_Attention_90220083019846.

Multi-head attention block: q/k/v = X@W{q,k,v}, scores = q@k^T + cb@k^T
(content bias folded into q), softmax, O = P@v, Z = X + O@Wo^T + b, LayerNorm.

Sharding over 8 NeuronCores: data-parallel over batch (2 groups of 4 cores) x
tensor-parallel over heads (4 heads per core). Output projection partial sums
are combined with a chunked ReduceScatter within each batch group; residual +
LayerNorm run on the scattered shards.

Dataflow is fully "transposed": the host passes X^T, so every matmul contracts
over the partition axis with no on-device transposes. Matmuls run in bf16
(f32 PSUM accumulation); softmax sums come free from the P@v matmul via a
fused ones-column in v (M=65). PSUM is organized as two 4-bank slots: score
tiles, their exp-consumers, the P@v partial accumulators and all projection
accumulations rotate through the same two slots, with cross-group O
accumulation done in SBUF by the vector engine.
"""

import contextlib
import ctypes
import sys
import types

sys.path.insert(0, "/opt/trn_rl_repo")

import numpy as np

# ---------------------------------------------------------------- profile hook
# The agent image's antenv lacks axon_hooks; provide it so that
# run_bass_kernel_spmd(trace=True) / BASS_TRACE=1 can capture NTFF profiles.
def _install_profile_hook():
    if "antenv.axon_hooks" in sys.modules:
        return
    try:
        import antenv
    except ImportError:
        return
    mod = types.ModuleType("antenv.axon_hooks")
    mod._hook = None
    mod.set_axon_ntff_profile_hook = lambda h: setattr(mod, "_hook", h)
    mod.get_axon_ntff_profile_hook = lambda: mod._hook
    sys.modules["antenv.axon_hooks"] = mod
    antenv.axon_hooks = mod
    try:
        lib = ctypes.CDLL("/opt/axon/libaxon_pjrt.so")
        if not hasattr(lib, "axon_start_nrt_profile"):
            return
        lib.axon_start_nrt_profile.argtypes = [
            ctypes.POINTER(ctypes.c_int64),
            ctypes.c_size_t,
        ]
        lib.axon_start_nrt_profile.restype = ctypes.c_int64
        lib.axon_stop_nrt_profile.argtypes = [ctypes.c_char_p]
        lib.axon_stop_nrt_profile.restype = ctypes.c_int64

        @contextlib.contextmanager
        def _hook(output_dir, device_ids):
            import jax

            jax.devices()
            if device_ids:
                ids = (ctypes.c_int64 * len(device_ids))(*device_ids)
                rc = lib.axon_start_nrt_profile(ids, len(device_ids))
            else:
                rc = lib.axon_start_nrt_profile(None, 0)
            if rc != 0:
                raise RuntimeError(f"axon_start_nrt_profile rc={rc}")
            try:
                yield
            finally:
                n = lib.axon_stop_nrt_profile(str(output_dir).encode())
                print(f"profile: {n} file(s) written to {output_dir}", file=sys.stderr)

        mod.set_axon_ntff_profile_hook(_hook)
    except OSError:
        pass


_install_profile_hook()

# ------------------------------------------------------------------- constants
B, L, D, H, HD = 2, 2048, 1024, 16, 64
NCORES = 8
GROUP = 4            # cores per batch group (tensor-parallel over heads)
HL = H // GROUP      # local heads per core
DL = HL * HD         # local head dims per core
QB = 512             # query block (tokens per pipeline chunk)
NQB = L // QB
RG = [[0, 1, 2, 3], [4, 5, 6, 7]]
LN_EPS = 1e-5
RSQRT_MAGIC = 0x5F3759DF

_PROGRAM = None
LAST_RESULT = None


def _build_program():
    import concourse.tile as tile
    from concourse import bacc, mybir

    fr = mybir.dt.float32r
    f32 = mybir.dt.float32
    bf16 = mybir.dt.bfloat16
    i32 = mybir.dt.int32
    Exp = mybir.ActivationFunctionType.Exp
    Alu = mybir.AluOpType

    nc = bacc.Bacc("TRN2", target_bir_lowering=False, debug=False,
                   num_devices=NCORES)

    xt_d = nc.dram_tensor("xt", (D, L), bf16, kind="ExternalInput").ap()
    wq_d = nc.dram_tensor("wq", (D, DL), bf16, kind="ExternalInput").ap()
    wk_d = nc.dram_tensor("wk", (D, DL), bf16, kind="ExternalInput").ap()
    wv_d = nc.dram_tensor("wv", (D, DL), bf16, kind="ExternalInput").ap()
    wot_d = nc.dram_tensor("wot", (DL, D), bf16, kind="ExternalInput").ap()
    cb_d = nc.dram_tensor("cb", (DL, 1), f32, kind="ExternalInput").ap()
    xres_d = nc.dram_tensor("xres", (QB, D), f32, kind="ExternalInput").ap()
    wob_d = nc.dram_tensor("wob", (1, D), f32, kind="ExternalInput").ap()
    lng_d = nc.dram_tensor("lng", (1, D), f32, kind="ExternalInput").ap()
    lnb_d = nc.dram_tensor("lnb", (1, D), f32, kind="ExternalInput").ap()
    ones_d = nc.dram_tensor("ones", (128, 128), fr, kind="ExternalInput").ap()
    vones_d = nc.dram_tensor("vones", (128, (L // 128) * HL), bf16,
                             kind="ExternalInput").ap()
    out_d = nc.dram_tensor("out", (QB, D), f32, kind="ExternalOutput").ap()

    NCH = 2 * NQB   # RS chunks (half a query block each)
    ccin = [nc.dram_tensor(f"ccin{t}", (QB // 2, D), f32, kind="Internal").ap()
            for t in range(NCH)]
    ccout = [nc.dram_tensor(f"ccout{t}", (QB // 2 // GROUP, D), f32,
                            kind="Internal").ap()
             for t in range(NCH)]

    with tile.TileContext(nc) as tc, contextlib.ExitStack() as ctx:
        # ---------------- persistent pools
        wp = ctx.enter_context(tc.tile_pool(name="wp", bufs=1))
        kqv = ctx.enter_context(tc.tile_pool(name="kqv", bufs=1))
        cons = ctx.enter_context(tc.tile_pool(name="cons", bufs=1))
        # two 3-bank psum slots for scores/projections + a dedicated P@v
        # accumulator pool (2 banks); the sums-broadcast rotates through psp
        psp = ctx.enter_context(tc.tile_pool(name="psp", bufs=2, space="PSUM"))
        ohp = ctx.enter_context(tc.tile_pool(name="ohp", bufs=1, space="PSUM"))
        rbp = ctx.enter_context(tc.tile_pool(name="rbp", bufs=1, space="PSUM"))

        wq_t = wp.tile([128, 8, DL], bf16)
        wk_t = wp.tile([128, 8, DL], bf16)
        wv_t = wp.tile([128, 8, DL], bf16)
        wot_t = wp.tile([128, HL, D], bf16)
        nc.sync.dma_start(out=wk_t, in_=wk_d.rearrange("(c p) o -> p c o", p=128))
        nc.sync.dma_start(out=wq_t, in_=wq_d.rearrange("(c p) o -> p c o", p=128))
        nc.sync.dma_start(out=wv_t, in_=wv_d.rearrange("(c p) o -> p c o", p=128))
        # rows 64..127 of each wot chunk are zero: they pair with the zero
        # rows of the padded ohn stationary operand (full-square matmuls
        # stream 2x faster than K<128 ones)
        nc.gpsimd.memset(wot_t[64:128, :, :], 0.0)
        nc.sync.dma_start(out=wot_t[0:64, :, :],
                          in_=wot_d.rearrange("(h p) e -> p h e", p=64))

        # k^T with the other head of the pair zeroed (full-square lhsT);
        # q^T keeps both heads (zero weights ignore the other head's rows)
        kt_ev = kqv.tile([128, 2, L], bf16)
        kt_od = kqv.tile([128, 2, L], bf16)
        qt = kqv.tile([128, 2, L], bf16)     # q^T (+cb)
        vaug = kqv.tile([128, L // 128, HL * 128], bf16)  # v | ones | zeros
        ohn = kqv.tile([128, HL, L], bf16)   # normalized Oh^T (rows 64+ zero)
        nc.gpsimd.memset(kt_ev[64:128, :, :], 0.0)
        nc.gpsimd.memset(kt_od[0:64, :, :], 0.0)
        nc.gpsimd.memset(vaug, 0.0)
        nc.gpsimd.memset(ohn[64:128, :, :], 0.0)

        cb_t = cons.tile([128, 2], f32)
        nc.sync.dma_start(out=cb_t, in_=cb_d.rearrange("(m p) x -> p (m x)", p=128))
        # lhsT for the sums broadcast: row 64 ones, all else zero (f32r)
        ones_t = cons.tile([128, 128], fr)
        nc.sync.dma_start(out=ones_t, in_=ones_d)
        wob_t = cons.tile([128, D], f32)
        nc.sync.dma_start(out=wob_t, in_=wob_d.partition_broadcast(128))
        lng_t = cons.tile([128, D], f32)
        nc.sync.dma_start(out=lng_t, in_=lng_d.partition_broadcast(128))
        lnb_t = cons.tile([128, D], f32)
        nc.sync.dma_start(out=lnb_t, in_=lnb_d.partition_broadcast(128))
        magic_t = cons.tile([128, 1], i32)
        nc.vector.memset(magic_t, RSQRT_MAGIC)

        nc.sync.dma_start(
            out=vaug.rearrange("p k (h x) -> p k h x", h=HL)[:, :, :, HD:HD + 1],
            in_=vones_d.rearrange("p (k h x) -> p k h x", k=L // 128, h=HL),
        )

        # ---------------- stage A: projections (needs X^T)
        with tc.tile_pool(name="xtp", bufs=1) as xtp:
            xt = xtp.tile([128, 8, L], bf16)
            for c in range(8):
                nc.sync.dma_start(out=xt[:, c, :], in_=xt_d[128 * c:128 * (c + 1), :])

            # k^T / q^T: out-dims on partitions, tokens free
            for m in range(2):
                for t4 in range(NQB):
                    tsl = slice(QB * t4, QB * (t4 + 1))
                    for w_t, is_q in ((wk_t, False), (wq_t, True)):
                        ps = psp.tile([128, 3 * 512], f32, tag="ps")
                        for c in range(8):
                            nc.tensor.matmul(
                                out=ps[:, 0:512],
                                lhsT=w_t[:, c, 128 * m:128 * (m + 1)],
                                rhs=xt[:, c, tsl],
                                start=(c == 0), stop=(c == 7),
                            )
                        if is_q:
                            nc.vector.tensor_scalar_add(
                                out=qt[:, m, tsl], in0=ps[:, 0:512],
                                scalar1=cb_t[:, m:m + 1])
                        else:
                            nc.vector.tensor_copy(out=kt_ev[0:64, m, tsl],
                                                  in_=ps[0:64, 0:512])
                            nc.vector.tensor_copy(out=kt_od[64:128, m, tsl],
                                                  in_=ps[64:128, 0:512])

            # v: tokens on partitions, head dims free (augmented with ones col)
            for kc in range(L // 128):
                ps = psp.tile([128, 3 * 512], f32, tag="ps")
                for c in range(8):
                    nc.tensor.matmul(
                        out=ps[:, 0:DL],
                        lhsT=xt[:, c, 128 * kc:128 * (kc + 1)],
                        rhs=wv_t[:, c, :],
                        start=(c == 0), stop=(c == 7),
                    )
                nc.vector.tensor_copy(
                    out=vaug[:, kc, :].rearrange("p (h x) -> p h x", h=HL)[:, :, 0:HD],
                    in_=ps[:, 0:DL].rearrange("p (h x) -> p h x", h=HL),
                )
                # vaug columns 64 (ones) and 65..127 (zeros) were set up front

        # ---------------- stage B (attention) + stage C (proj/RS/LN)
        with tc.tile_pool(name="ptp", bufs=3) as ptp, \
             tc.tile_pool(name="ohsp", bufs=2) as ohsp, \
             tc.tile_pool(name="recp", bufs=2) as recp, \
             tc.tile_pool(name="zevp", bufs=2) as zevp, \
             tc.tile_pool(name="lnp", bufs=2) as lnp:

            groups = [(0, 3), (3, 3), (6, 3), (9, 3), (12, 3), (15, 1)]
            nkc = L // 128

            for qb in range(NQB):
                # ---- attention for all local heads on this query block
                for h in range(HL):
                    mi = h // 2
                    ktp = kt_ev if h % 2 == 0 else kt_od
                    qT_b = qt[:, mi, QB * qb:QB * (qb + 1)]
                    oh = ohp.tile([128, 512], f32, tag="oh")
                    for kc0, n in groups:
                        st = psp.tile([128, 3 * 512], f32, tag="ps")
                        for i in range(n):
                            kc = kc0 + i
                            nc.tensor.matmul(
                                out=st[:, 512 * i:512 * (i + 1)],
                                lhsT=ktp[:, mi, 128 * kc:128 * (kc + 1)],
                                rhs=qT_b,
                                start=True, stop=True,
                            )
                        pt = ptp.tile([128, 3 * 512], bf16, tag="pt")
                        nc.scalar.activation(out=pt[:, :512 * n],
                                             in_=st[:, :512 * n], func=Exp)
                        for i in range(n):
                            kc = kc0 + i
                            nc.tensor.matmul(
                                out=oh,
                                lhsT=vaug[:, kc, 128 * h:128 * (h + 1)],
                                rhs=pt[:, 512 * i:512 * (i + 1)],
                                start=(kc == 0), stop=(kc == nkc - 1),
                            )
                    # evacuate Oh + sums, broadcast sums, reciprocal, scale
                    ohs = ohsp.tile([65, 512], fr, tag="ohs")
                    with nc.allow_low_precision(reason="f32r rounding of Oh"):
                        nc.vector.tensor_copy(out=ohs, in_=oh[0:65, :])
                    rb = rbp.tile([128, 512], f32, tag="rb")
                    nc.tensor.matmul(out=rb, lhsT=ones_t[0:65, :],
                                     rhs=ohs, start=True, stop=True)
                    rec = recp.tile([64, 512], f32, tag="rec")
                    nc.vector.reciprocal_approx_fast(out=rec, in_=rb[0:64, :])
                    nc.vector.tensor_mul(
                        out=ohn[0:64, h, QB * qb:QB * (qb + 1)],
                        in0=ohs[0:64, :], in1=rec)

                # ---- output projection partial for this token block
                for tcl in range(QB // 128):
                    t0 = QB * qb + 128 * tcl
                    zev = zevp.tile([128, D], f32)
                    for ec in range(2):
                        zp = psp.tile([128, 3 * 512], f32, tag="ps")
                        for h in range(HL):
                            nc.tensor.matmul(
                                out=zp[:, 0:512],
                                lhsT=ohn[:, h, t0:t0 + 128],
                                rhs=wot_t[:, h, 512 * ec:512 * (ec + 1)],
                                start=(h == 0), stop=(h == HL - 1),
                            )
                        nc.vector.tensor_copy(out=zev[:, 512 * ec:512 * (ec + 1)],
                                              in_=zp[:, 0:512])
                    ch = 2 * qb + tcl // 2
                    nc.sync.dma_start(
                        out=ccin[ch][128 * (tcl % 2):128 * (tcl % 2 + 1), :],
                        in_=zev)
                    # combine partials across the batch group per half-block
                    if tcl % 2 == 1:
                        nc.gpsimd.collective_compute(
                            "ReduceScatter", Alu.add,
                            ins=[ccin[ch][:]], outs=[ccout[ch][:]],
                            replica_groups=RG,
                        )

            # ---- residual + bias + LayerNorm, deferred so the in-order DVE
            # stream never blocks attention work behind a ReduceScatter wait
            for qb in range(NQB):
              with tc.tile_wait_until(0.16 + 0.04 * qb):
                zt = lnp.tile([128, D], f32, tag="zt")
                nc.sync.dma_start(out=zt[0:64, :], in_=ccout[2 * qb])
                nc.sync.dma_start(out=zt[64:128, :], in_=ccout[2 * qb + 1])
                xr = lnp.tile([128, D], f32, tag="xr")
                nc.sync.dma_start(out=xr, in_=xres_d[128 * qb:128 * (qb + 1), :])
                nc.vector.tensor_add(out=zt, in0=zt, in1=xr)
                nc.vector.tensor_add(out=zt, in0=zt, in1=wob_t)

                stats = lnp.tile([128, 2, 6], f32, tag="stats")
                for sg in range(2):
                    nc.vector.bn_stats(out=stats[:, sg, :],
                                       in_=zt[:, 512 * sg:512 * (sg + 1)])
                mv = lnp.tile([128, 2], f32, tag="mv")
                nc.vector.bn_aggr(out=mv, in_=stats)

                # rstd = rsqrt(var + eps), DVE-only (avoids ACT table thrash)
                ve = lnp.tile([128, 1], f32, tag="ve")
                nc.vector.tensor_scalar_add(out=ve, in0=mv[:, 1:2], scalar1=LN_EPS)
                y = lnp.tile([128, 1], f32, tag="y")
                nc.vector.tensor_scalar(
                    out=y.bitcast(i32), in0=ve.bitcast(i32), scalar1=1,
                    scalar2=None, op0=Alu.logical_shift_right)
                nc.vector.tensor_sub(out=y.bitcast(i32), in0=magic_t,
                                     in1=y.bitcast(i32))
                tnw = lnp.tile([128, 1], f32, tag="tnw")
                for _ in range(3):
                    nc.vector.tensor_mul(out=tnw, in0=ve, in1=y)
                    nc.vector.tensor_mul(out=tnw, in0=tnw, in1=y)
                    nc.vector.tensor_scalar(out=tnw, in0=tnw, scalar1=-0.5,
                                            scalar2=1.5, op0=Alu.mult, op1=Alu.add)
                    nc.vector.tensor_mul(out=y, in0=y, in1=tnw)

                nc.vector.tensor_scalar(out=zt, in0=zt, scalar1=mv[:, 0:1],
                                        scalar2=y, op0=Alu.subtract, op1=Alu.mult)
                nc.vector.tensor_mul(out=zt, in0=zt, in1=lng_t)
                nc.vector.tensor_add(out=zt, in0=zt, in1=lnb_t)
                nc.sync.dma_start(out=out_d[128 * qb:128 * (qb + 1), :], in_=zt)


    nc.compile()
    return nc


def _get_program():
    global _PROGRAM
    if _PROGRAM is None:
        _PROGRAM = _build_program()
    return _PROGRAM


def kernel(X, Y, Wq, Wk, Wv, cb, Wo_w, Wo_b, ln_g, ln_b):
    import ml_dtypes
    from concourse import bass_utils

    prog = _get_program()
    bf = ml_dtypes.bfloat16

    X = np.asarray(X, dtype=np.float32)
    Wq = np.asarray(Wq, dtype=np.float32)
    Wk = np.asarray(Wk, dtype=np.float32)
    Wv = np.asarray(Wv, dtype=np.float32)
    cb = np.asarray(cb, dtype=np.float32)
    Wo_w = np.asarray(Wo_w, dtype=np.float32)
    Wo_b = np.asarray(Wo_b, dtype=np.float32)
    ln_g = np.asarray(ln_g, dtype=np.float32)
    ln_b = np.asarray(ln_b, dtype=np.float32)

    WoT = np.ascontiguousarray(Wo_w.T)
    ones_arr = np.zeros((128, 128), np.float32)
    ones_arr[64, :] = 1.0
    in_maps = []
    for c in range(NCORES):
        b, hp, r = c // GROUP, c % GROUP, c % GROUP
        Xb = X[b]
        rows = np.concatenate(
            [np.arange(256 * c + 64 * r, 256 * c + 64 * r + 64)
             for c in range(2 * NQB)])
        csl = slice(DL * hp, DL * (hp + 1))
        in_maps.append({
            "xt": np.ascontiguousarray(Xb.T).astype(bf),
            "xres": np.ascontiguousarray(Xb[rows]),
            "wq": np.ascontiguousarray(Wq[:, csl]).astype(bf),
            "wk": np.ascontiguousarray(Wk[:, csl]).astype(bf),
            "wv": np.ascontiguousarray(Wv[:, csl]).astype(bf),
            "wot": np.ascontiguousarray(WoT[csl, :]).astype(bf),
            "cb": np.ascontiguousarray(cb[csl].reshape(DL, 1)),
            "wob": np.ascontiguousarray(Wo_b.reshape(1, D)),
            "lng": np.ascontiguousarray(ln_g.reshape(1, D)),
            "lnb": np.ascontiguousarray(ln_b.reshape(1, D)),
            "ones": ones_arr,
            "vones": np.ones((128, (L // 128) * HL), bf),
        })

    res = bass_utils.run_bass_kernel_spmd(prog, in_maps, core_ids=list(range(NCORES)))
    global LAST_RESULT
    LAST_RESULT = res

    out = np.empty((B, L, D), np.float32)
    for cid in range(NCORES):
        b, r = cid // GROUP, cid % GROUP
        o = res.results[cid]["out"]
        for c in range(2 * NQB):
            out[b, 256 * c + 64 * r:256 * c + 64 * r + 64] = o[64 * c:64 * (c + 1)]
    return out


if __name__ == "__main__":
    rng = np.random.default_rng(0)
    ins = {
        "X": rng.standard_normal((B, L, D)).astype(np.float32),
        "Y": rng.standard_normal((B, L, D)).astype(np.float32),
        "Wq": (rng.uniform(-1, 1, (D, D)) / 32).astype(np.float32),
        "Wk": (rng.uniform(-1, 1, (D, D)) / 32).astype(np.float32),
        "Wv": (rng.uniform(-1, 1, (D, D)) / 32).astype(np.float32),
        "cb": np.zeros(D, np.float32),
        "Wo_w": (rng.uniform(-1, 1, (D, D)) / 32).astype(np.float32),
        "Wo_b": (rng.uniform(-1, 1, D) / 32).astype(np.float32),
        "ln_g": np.ones(D, np.float32),
        "ln_b": np.zeros(D, np.float32),
    }
    out = kernel(**ins)
    print("out", out.shape, out.dtype, float(np.abs(out).max()))
    print("exec_time_ns:", LAST_RESULT.exec_time_ns)


# revision 33
# speedup vs baseline: 1.1449x; 1.0418x over previous
"""Trainium2 Bass kernel for nn_Attention_90220083019846.

Multi-head attention block: q/k/v = X@W{q,k,v}, scores = q@k^T + cb@k^T
(content bias folded into q), softmax, O = P@v, Z = X + O@Wo^T + b, LayerNorm.

Sharding over 8 NeuronCores: data-parallel over batch (2 groups of 4 cores) x
tensor-parallel over heads (4 heads per core). Output projection partial sums
are combined with a chunked ReduceScatter within each batch group; residual +
LayerNorm run on the scattered shards.

Dataflow is fully "transposed": the host passes X^T, so every matmul contracts
over the partition axis with no on-device transposes. Matmuls run in bf16
(f32 PSUM accumulation); softmax sums come free from the P@v matmul via a
fused ones-column in v (M=65). PSUM is organized as two 4-bank slots: score
tiles, their exp-consumers, the P@v partial accumulators and all projection
accumulations rotate through the same two slots, with cross-group O
accumulation done in SBUF by the vector engine.
"""

import contextlib
import ctypes
import sys
import types

sys.path.insert(0, "/opt/trn_rl_repo")

import numpy as np

# ---------------------------------------------------------------- profile hook
# The agent image's antenv lacks axon_hooks; provide it so that
# run_bass_kernel_spmd(trace=True) / BASS_TRACE=1 can capture NTFF profiles.
def _install_profile_hook():
    if "antenv.axon_hooks" in sys.modules:
        return
    try:
        import antenv
    except ImportError:
        return
    mod = types.ModuleType("antenv.axon_hooks")
    mod._hook = None
    mod.set_axon_ntff_profile_hook = lambda h: setattr(mod, "_hook", h)
    mod.get_axon_ntff_profile_hook = lambda: mod._hook
    sys.modules["antenv.axon_hooks"] = mod
    antenv.axon_hooks = mod
    try:
        lib = ctypes.CDLL("/opt/axon/libaxon_pjrt.so")
        if not hasattr(lib, "axon_start_nrt_profile"):
            return
        lib.axon_start_nrt_profile.argtypes = [
            ctypes.POINTER(ctypes.c_int64),
            ctypes.c_size_t,
        ]
        lib.axon_start_nrt_profile.restype = ctypes.c_int64
        lib.axon_stop_nrt_profile.argtypes = [ctypes.c_char_p]
        lib.axon_stop_nrt_profile.restype = ctypes.c_int64

        @contextlib.contextmanager
        def _hook(output_dir, device_ids):
            import jax

            jax.devices()
            if device_ids:
                ids = (ctypes.c_int64 * len(device_ids))(*device_ids)
                rc = lib.axon_start_nrt_profile(ids, len(device_ids))
            else:
                rc = lib.axon_start_nrt_profile(None, 0)
            if rc != 0:
                raise RuntimeError(f"axon_start_nrt_profile rc={rc}")
            try:
                yield
            finally:
                n = lib.axon_stop_nrt_profile(str(output_dir).encode())
                print(f"profile: {n} file(s) written to {output_dir}", file=sys.stderr)

        mod.set_axon_ntff_profile_hook(_hook)
    except OSError:
        pass


_install_profile_hook()

# ------------------------------------------------------------------- constants
B, L, D, H, HD = 2, 2048, 1024, 16, 64
NCORES = 8
GROUP = 4            # cores per batch group (tensor-parallel over heads)
HL = H // GROUP      # local heads per core
DL = HL * HD         # local head dims per core
QB = 512             # query block (tokens per pipeline chunk)
NQB = L // QB
RG = [[0, 1, 2, 3], [4, 5, 6, 7]]
LN_EPS = 1e-5
RSQRT_MAGIC = 0x5F3759DF

_PROGRAM = None
LAST_RESULT = None


def _build_program():
    import concourse.tile as tile
    from concourse import bacc, mybir

    fr = mybir.dt.float32r
    f32 = mybir.dt.float32
    bf16 = mybir.dt.bfloat16
    i32 = mybir.dt.int32
    Exp = mybir.ActivationFunctionType.Exp
    Alu = mybir.AluOpType

    nc = bacc.Bacc("TRN2", target_bir_lowering=False, debug=False,
                   num_devices=NCORES)

    xt_d = nc.dram_tensor("xt", (D, L), bf16, kind="ExternalInput").ap()
    wq_d = nc.dram_tensor("wq", (D, DL), bf16, kind="ExternalInput").ap()
    wk_d = nc.dram_tensor("wk", (D, DL), bf16, kind="ExternalInput").ap()
    wv_d = nc.dram_tensor("wv", (D, DL), bf16, kind="ExternalInput").ap()
    wot_d = nc.dram_tensor("wot", (DL, D), bf16, kind="ExternalInput").ap()
    cb_d = nc.dram_tensor("cb", (DL, 1), f32, kind="ExternalInput").ap()
    xres_d = nc.dram_tensor("xres", (QB, D), f32, kind="ExternalInput").ap()
    wob_d = nc.dram_tensor("wob", (1, D), f32, kind="ExternalInput").ap()
    lng_d = nc.dram_tensor("lng", (1, D), f32, kind="ExternalInput").ap()
    lnb_d = nc.dram_tensor("lnb", (1, D), f32, kind="ExternalInput").ap()
    ones_d = nc.dram_tensor("ones", (128, 128), fr, kind="ExternalInput").ap()
    vones_d = nc.dram_tensor("vones", (128, (L // 128) * HL), bf16,
                             kind="ExternalInput").ap()
    out_d = nc.dram_tensor("out", (QB, D), f32, kind="ExternalOutput").ap()

    NCH = 2 * NQB   # RS chunks (half a query block each)
    ccin = [nc.dram_tensor(f"ccin{t}", (QB // 2, D), f32, kind="Internal").ap()
            for t in range(NCH)]
    ccout = [nc.dram_tensor(f"ccout{t}", (QB // 2 // GROUP, D), f32,
                            kind="Internal").ap()
             for t in range(NCH)]

    with tile.TileContext(nc) as tc, contextlib.ExitStack() as ctx:
        # ---------------- persistent pools
        wp = ctx.enter_context(tc.tile_pool(name="wp", bufs=1))
        kqv = ctx.enter_context(tc.tile_pool(name="kqv", bufs=1))
        cons = ctx.enter_context(tc.tile_pool(name="cons", bufs=1))
        # two 3-bank psum slots for scores/projections + a dedicated P@v
        # accumulator pool (2 banks); the sums-broadcast rotates through psp
        psp = ctx.enter_context(tc.tile_pool(name="psp", bufs=2, space="PSUM"))
        ohp = ctx.enter_context(tc.tile_pool(name="ohp", bufs=1, space="PSUM"))
        rbp = ctx.enter_context(tc.tile_pool(name="rbp", bufs=1, space="PSUM"))

        wq_t = wp.tile([128, 8, DL], bf16)
        wk_t = wp.tile([128, 8, DL], bf16)
        wv_t = wp.tile([128, 8, DL], bf16)
        wot_t = wp.tile([128, HL, D], bf16)
        nc.sync.dma_start(out=wk_t, in_=wk_d.rearrange("(c p) o -> p c o", p=128))

        # k^T with the other head of the pair zeroed (full-square lhsT);
        # q^T keeps both heads (zero weights ignore the other head's rows)
        kt_ev = kqv.tile([128, 2, L], bf16)
        kt_od = kqv.tile([128, 2, L], bf16)
        qt = kqv.tile([128, 2, L], bf16)     # q^T (+cb)
        vaug = kqv.tile([128, L // 128, HL * 128], bf16)  # v | ones | zeros
        ohn = kqv.tile([128, HL, L], bf16)   # normalized Oh^T (rows 64+ zero)
        nc.gpsimd.memset(kt_ev[64:128, :, :], 0.0)
        nc.gpsimd.memset(kt_od[0:64, :, :], 0.0)
        nc.gpsimd.memset(vaug, 0.0)
        nc.gpsimd.memset(ohn[64:128, :, :], 0.0)

        cb_t = cons.tile([128, 2], f32)
        nc.sync.dma_start(out=cb_t, in_=cb_d.rearrange("(m p) x -> p (m x)", p=128))
        # lhsT for the sums broadcast: row 64 ones, all else zero (f32r)
        ones_t = cons.tile([128, 128], fr)
        nc.sync.dma_start(out=ones_t, in_=ones_d)
        wob_t = cons.tile([128, D], f32)
        nc.sync.dma_start(out=wob_t, in_=wob_d.partition_broadcast(128))
        lng_t = cons.tile([128, D], f32)
        nc.sync.dma_start(out=lng_t, in_=lng_d.partition_broadcast(128))
        lnb_t = cons.tile([128, D], f32)
        nc.sync.dma_start(out=lnb_t, in_=lnb_d.partition_broadcast(128))
        magic_t = cons.tile([128, 1], i32)
        nc.vector.memset(magic_t, RSQRT_MAGIC)

        nc.sync.dma_start(
            out=vaug.rearrange("p k (h x) -> p k h x", h=HL)[:, :, :, HD:HD + 1],
            in_=vones_d.rearrange("p (k h x) -> p k h x", k=L // 128, h=HL),
        )

        # ---------------- stage A: projections (needs X^T)
        with tc.tile_pool(name="xtp", bufs=1) as xtp:
            xt = xtp.tile([128, 8, L], bf16)
            for c in range(8):
                nc.sync.dma_start(out=xt[:, c, :], in_=xt_d[128 * c:128 * (c + 1), :])
            nc.sync.dma_start(out=wq_t, in_=wq_d.rearrange("(c p) o -> p c o", p=128))
            nc.sync.dma_start(out=wv_t, in_=wv_d.rearrange("(c p) o -> p c o", p=128))
            # rows 64..127 of each wot chunk are zero: they pair with the zero
            # rows of the padded ohn stationary operand (full-square matmuls
            # stream 2x faster than K<128 ones)
            nc.gpsimd.memset(wot_t[64:128, :, :], 0.0)
            nc.sync.dma_start(out=wot_t[0:64, :, :],
                              in_=wot_d.rearrange("(h p) e -> p h e", p=64))

            # k^T / q^T: out-dims on partitions, tokens free
            for m in range(2):
                for t4 in range(NQB):
                    tsl = slice(QB * t4, QB * (t4 + 1))
                    for w_t, is_q in ((wk_t, False), (wq_t, True)):
                        ps = psp.tile([128, 3 * 512], f32, tag="ps")
                        for c in range(8):
                            nc.tensor.matmul(
                                out=ps[:, 0:512],
                                lhsT=w_t[:, c, 128 * m:128 * (m + 1)],
                                rhs=xt[:, c, tsl],
                                start=(c == 0), stop=(c == 7),
                            )
                        if is_q:
                            nc.vector.tensor_scalar_add(
                                out=qt[:, m, tsl], in0=ps[:, 0:512],
                                scalar1=cb_t[:, m:m + 1])
                        else:
                            nc.vector.tensor_copy(out=kt_ev[0:64, m, tsl],
                                                  in_=ps[0:64, 0:512])
                            nc.vector.tensor_copy(out=kt_od[64:128, m, tsl],
                                                  in_=ps[64:128, 0:512])

            # v: tokens on partitions, head dims free (augmented with ones col)
            for kc in range(L // 128):
                ps = psp.tile([128, 3 * 512], f32, tag="ps")
                for c in range(8):
                    nc.tensor.matmul(
                        out=ps[:, 0:DL],
                        lhsT=xt[:, c, 128 * kc:128 * (kc + 1)],
                        rhs=wv_t[:, c, :],
                        start=(c == 0), stop=(c == 7),
                    )
                nc.vector.tensor_copy(
                    out=vaug[:, kc, :].rearrange("p (h x) -> p h x", h=HL)[:, :, 0:HD],
                    in_=ps[:, 0:DL].rearrange("p (h x) -> p h x", h=HL),
                )
                # vaug columns 64 (ones) and 65..127 (zeros) were set up front

        # ---------------- stage B (attention) + stage C (proj/RS/LN)
        with tc.tile_pool(name="ptp", bufs=3) as ptp, \
             tc.tile_pool(name="ohsp", bufs=2) as ohsp, \
             tc.tile_pool(name="recp", bufs=2) as recp, \
             tc.tile_pool(name="zevp", bufs=2) as zevp, \
             tc.tile_pool(name="lnp", bufs=2) as lnp:

            groups = [(0, 3), (3, 3), (6, 3), (9, 3), (12, 3), (15, 1)]
            nkc = L // 128

            for qb in range(NQB):
                # ---- attention for all local heads on this query block
                for h in range(HL):
                    mi = h // 2
                    ktp = kt_ev if h % 2 == 0 else kt_od
                    qT_b = qt[:, mi, QB * qb:QB * (qb + 1)]
                    oh = ohp.tile([128, 512], f32, tag="oh")
                    for kc0, n in groups:
                        st = psp.tile([128, 3 * 512], f32, tag="ps")
                        for i in range(n):
                            kc = kc0 + i
                            nc.tensor.matmul(
                                out=st[:, 512 * i:512 * (i + 1)],
                                lhsT=ktp[:, mi, 128 * kc:128 * (kc + 1)],
                                rhs=qT_b,
                                start=True, stop=True,
                            )
                        pt = ptp.tile([128, 3 * 512], bf16, tag="pt")
                        nc.scalar.activation(out=pt[:, :512 * n],
                                             in_=st[:, :512 * n], func=Exp)
                        for i in range(n):
                            kc = kc0 + i
                            nc.tensor.matmul(
                                out=oh,
                                lhsT=vaug[:, kc, 128 * h:128 * (h + 1)],
                                rhs=pt[:, 512 * i:512 * (i + 1)],
                                start=(kc == 0), stop=(kc == nkc - 1),
                            )
                    # evacuate Oh + sums, broadcast sums, reciprocal, scale
                    ohs = ohsp.tile([65, 512], fr, tag="ohs")
                    with nc.allow_low_precision(reason="f32r rounding of Oh"):
                        nc.vector.tensor_copy(out=ohs, in_=oh[0:65, :])
                    rb = rbp.tile([128, 512], f32, tag="rb")
                    nc.tensor.matmul(out=rb, lhsT=ones_t[0:65, :],
                                     rhs=ohs, start=True, stop=True)
                    rec = recp.tile([64, 512], f32, tag="rec")
                    nc.vector.reciprocal_approx_fast(out=rec, in_=rb[0:64, :])
                    nc.vector.tensor_mul(
                        out=ohn[0:64, h, QB * qb:QB * (qb + 1)],
                        in0=ohs[0:64, :], in1=rec)

                # ---- output projection partial for this token block
                for tcl in range(QB // 128):
                    t0 = QB * qb + 128 * tcl
                    zev = zevp.tile([128, D], f32)
                    for ec in range(2):
                        zp = psp.tile([128, 3 * 512], f32, tag="ps")
                        for h in range(HL):
                            nc.tensor.matmul(
                                out=zp[:, 0:512],
                                lhsT=ohn[:, h, t0:t0 + 128],
                                rhs=wot_t[:, h, 512 * ec:512 * (ec + 1)],
                                start=(h == 0), stop=(h == HL - 1),
                            )
                        nc.vector.tensor_copy(out=zev[:, 512 * ec:512 * (ec + 1)],
                                              in_=zp[:, 0:512])
                    ch = 2 * qb + tcl // 2
                    nc.sync.dma_start(
                        out=ccin[ch][128 * (tcl % 2):128 * (tcl % 2 + 1), :],
                        in_=zev)
                    # combine partials across the batch group per half-block
                    if tcl % 2 == 1:
                        nc.gpsimd.collective_compute(
                            "ReduceScatter", Alu.add,
                            ins=[ccin[ch][:]], outs=[ccout[ch][:]],
                            replica_groups=RG,
                        )

            # ---- residual + bias + LayerNorm, deferred so the in-order DVE
            # stream never blocks attention work behind a ReduceScatter wait
            for qb in range(NQB):
              with tc.tile_wait_until(0.21 + 0.03 * qb):
                zt = lnp.tile([128, D], f32, tag="zt")
                nc.sync.dma_start(out=zt[0:64, :], in_=ccout[2 * qb])
                nc.sync.dma_start(out=zt[64:128, :], in_=ccout[2 * qb + 1])
                xr = lnp.tile([128, D], f32, tag="xr")
                nc.sync.dma_start(out=xr, in_=xres_d[128 * qb:128 * (qb + 1), :])
                nc.vector.tensor_add(out=zt, in0=zt, in1=xr)
                nc.vector.tensor_add(out=zt, in0=zt, in1=wob_t)

                stats = lnp.tile([128, 2, 6], f32, tag="stats")
                for sg in range(2):
                    nc.vector.bn_stats(out=stats[:, sg, :],
                                       in_=zt[:, 512 * sg:512 * (sg + 1)])
                mv = lnp.tile([128, 2], f32, tag="mv")
                nc.vector.bn_aggr(out=mv, in_=stats)

                # rstd = rsqrt(var + eps), DVE-only (avoids ACT table thrash)
                ve = lnp.tile([128, 1], f32, tag="ve")
                nc.vector.tensor_scalar_add(out=ve, in0=mv[:, 1:2], scalar1=LN_EPS)
                y = lnp.tile([128, 1], f32, tag="y")
                nc.vector.tensor_scalar(
                    out=y.bitcast(i32), in0=ve.bitcast(i32), scalar1=1,
                    scalar2=None, op0=Alu.logical_shift_right)
                nc.vector.tensor_sub(out=y.bitcast(i32), in0=magic_t,
                                     in1=y.bitcast(i32))
                tnw = lnp.tile([128, 1], f32, tag="tnw")
                for _ in range(3):
                    nc.vector.tensor_mul(out=tnw, in0=ve, in1=y)
                    nc.vector.tensor_mul(out=tnw, in0=tnw, in1=y)
                    nc.vector.tensor_scalar(out=tnw, in0=tnw, scalar1=-0.5,
                                            scalar2=1.5, op0=Alu.mult, op1=Alu.add)
                    nc.vector.tensor_mul(out=y, in0=y, in1=tnw)

                nc.vector.tensor_scalar(out=zt, in0=zt, scalar1=mv[:, 0:1],
                                        scalar2=y, op0=Alu.subtract, op1=Alu.mult)
                nc.vector.tensor_mul(out=zt, in0=zt, in1=lng_t)
                nc.vector.tensor_add(out=zt, in0=zt, in1=lnb_t)
                nc.sync.dma_start(out=out_d[128 * qb:128 * (qb + 1), :], in_=zt)


    nc.compile()
    return nc


def _get_program():
    global _PROGRAM
    if _PROGRAM is None:
        _PROGRAM = _build_program()
    return _PROGRAM


def kernel(X, Y, Wq, Wk, Wv, cb, Wo_w, Wo_b, ln_g, ln_b):
    import ml_dtypes
    from concourse import bass_utils

    prog = _get_program()
    bf = ml_dtypes.bfloat16

    X = np.asarray(X, dtype=np.float32)
    Wq = np.asarray(Wq, dtype=np.float32)
    Wk = np.asarray(Wk, dtype=np.float32)
    Wv = np.asarray(Wv, dtype=np.float32)
    cb = np.asarray(cb, dtype=np.float32)
    Wo_w = np.asarray(Wo_w, dtype=np.float32)
    Wo_b = np.asarray(Wo_b, dtype=np.float32)
    ln_g = np.asarray(ln_g, dtype=np.float32)
    ln_b = np.asarray(ln_b, dtype=np.float32)

    WoT = np.ascontiguousarray(Wo_w.T)
    ones_arr = np.zeros((128, 128), np.float32)
    ones_arr[64, :] = 1.0
    in_maps = []
    for c in range(NCORES):
        b, hp, r = c // GROUP, c % GROUP, c % GROUP
        Xb = X[b]
        rows = np.concatenate(
            [np.arange(256 * c + 64 * r, 256 * c + 64 * r + 64)
             for c in range(2 * NQB)])
        csl = slice(DL * hp, DL * (hp + 1))
        in_maps.append({
            "xt": np.ascontiguousarray(Xb.T).astype(bf),
            "xres": np.ascontiguousarray(Xb[rows]),
            "wq": np.ascontiguousarray(Wq[:, csl]).astype(bf),
            "wk": np.ascontiguousarray(Wk[:, csl]).astype(bf),
            "wv": np.ascontiguousarray(Wv[:, csl]).astype(bf),
            "wot": np.ascontiguousarray(WoT[csl, :]).astype(bf),
            "cb": np.ascontiguousarray(cb[csl].reshape(DL, 1)),
            "wob": np.ascontiguousarray(Wo_b.reshape(1, D)),
            "lng": np.ascontiguousarray(ln_g.reshape(1, D)),
            "lnb": np.ascontiguousarray(ln_b.reshape(1, D)),
            "ones": ones_arr,
            "vones": np.ones((128, (L // 128) * HL), bf),
        })

    res = bass_utils.run_bass_kernel_spmd(prog, in_maps, core_ids=list(range(NCORES)))
    global LAST_RESULT
    LAST_RESULT = res

    out = np.empty((B, L, D), np.float32)
    for cid in range(NCORES):
        b, r = cid // GROUP, cid % GROUP
        o = res.results[cid]["out"]
        for c in range(2 * NQB):
            out[b, 256 * c + 64 * r:256 * c + 64 * r + 64] = o[64 * c:64 * (c + 1)]
    return out


if __name__ == "__main__":
    rng = np.random.default_rng(0)
    ins = {
        "X": rng.standard_normal((B, L, D)).astype(np.float32),
        "Y": rng.standard_normal((B, L, D)).astype(np.float32),
        "Wq": (rng.uniform(-1, 1, (D, D)) / 32).astype(np.float32),
        "Wk": (rng.uniform(-1, 1, (D, D)) / 32).astype(np.float32),
        "Wv": (rng.uniform(-1, 1, (D, D)) / 32).astype(np.float32),
        "cb": np.zeros(D, np.float32),
        "Wo_w": (rng.uniform(-1, 1, (D, D)) / 32).astype(np.float32),
        "Wo_b": (rng.uniform(-1, 1, D) / 32).astype(np.float32),
        "ln_g": np.ones(D, np.float32),
        "ln_b": np.zeros(D, np.float32),
    }
    out = kernel(**ins)
    print("out", out.shape, out.dtype, float(np.abs(out).max()))
    print("exec_time_ns:", LAST_RESULT.exec_time_ns)


# revision 34
# speedup vs baseline: 1.1638x; 1.0166x over previous
"""Trainium2 Bass kernel for nn_Attention_90220083019846.

Multi-head attention block: q/k/v = X@W{q,k,v}, scores = q@k^T + cb@k^T
(content bias folded into q), softmax, O = P@v, Z = X + O@Wo^T + b, LayerNorm.

Sharding over 8 NeuronCores: data-parallel over batch (2 groups of 4 cores) x
tensor-parallel over heads (4 heads per core). Output projection partial sums
are combined with a chunked ReduceScatter within each batch group; residual +
LayerNorm run on the scattered shards.

Dataflow is fully "transposed": the host passes X^T, so every matmul contracts
over the partition axis with no on-device transposes. Matmuls run in bf16
(f32 PSUM accumulation); softmax sums come free from the P@v matmul via a
fused ones-column in v (M=65). PSUM is organized as two 4-bank slots: score
tiles, their exp-consumers, the P@v partial accumulators and all projection
accumulations rotate through the same two slots, with cross-group O
accumulation done in SBUF by the vector engine.
"""

import contextlib
import ctypes
import sys
import types

sys.path.insert(0, "/opt/trn_rl_repo")

import numpy as np

# ---------------------------------------------------------------- profile hook
# The agent image's antenv lacks axon_hooks; provide it so that
# run_bass_kernel_spmd(trace=True) / BASS_TRACE=1 can capture NTFF profiles.
def _install_profile_hook():
    if "antenv.axon_hooks" in sys.modules:
        return
    try:
        import antenv
    except ImportError:
        return
    mod = types.ModuleType("antenv.axon_hooks")
    mod._hook = None
    mod.set_axon_ntff_profile_hook = lambda h: setattr(mod, "_hook", h)
    mod.get_axon_ntff_profile_hook = lambda: mod._hook
    sys.modules["antenv.axon_hooks"] = mod
    antenv.axon_hooks = mod
    try:
        lib = ctypes.CDLL("/opt/axon/libaxon_pjrt.so")
        if not hasattr(lib, "axon_start_nrt_profile"):
            return
        lib.axon_start_nrt_profile.argtypes = [
            ctypes.POINTER(ctypes.c_int64),
            ctypes.c_size_t,
        ]
        lib.axon_start_nrt_profile.restype = ctypes.c_int64
        lib.axon_stop_nrt_profile.argtypes = [ctypes.c_char_p]
        lib.axon_stop_nrt_profile.restype = ctypes.c_int64

        @contextlib.contextmanager
        def _hook(output_dir, device_ids):
            import jax

            jax.devices()
            if device_ids:
                ids = (ctypes.c_int64 * len(device_ids))(*device_ids)
                rc = lib.axon_start_nrt_profile(ids, len(device_ids))
            else:
                rc = lib.axon_start_nrt_profile(None, 0)
            if rc != 0:
                raise RuntimeError(f"axon_start_nrt_profile rc={rc}")
            try:
                yield
            finally:
                n = lib.axon_stop_nrt_profile(str(output_dir).encode())
                print(f"profile: {n} file(s) written to {output_dir}", file=sys.stderr)

        mod.set_axon_ntff_profile_hook(_hook)
    except OSError:
        pass


_install_profile_hook()

# ------------------------------------------------------------------- constants
B, L, D, H, HD = 2, 2048, 1024, 16, 64
NCORES = 8
GROUP = 4            # cores per batch group (tensor-parallel over heads)
HL = H // GROUP      # local heads per core
DL = HL * HD         # local head dims per core
QB = 512             # query block (tokens per pipeline chunk)
NQB = L // QB
RG = [[0, 1, 2, 3], [4, 5, 6, 7]]
LN_EPS = 1e-5
RSQRT_MAGIC = 0x5F3759DF

_PROGRAM = None
LAST_RESULT = None


def _build_program():
    import concourse.tile as tile
    from concourse import bacc, mybir

    fr = mybir.dt.float32r
    f32 = mybir.dt.float32
    bf16 = mybir.dt.bfloat16
    i32 = mybir.dt.int32
    Exp = mybir.ActivationFunctionType.Exp
    Alu = mybir.AluOpType

    nc = bacc.Bacc("TRN2", target_bir_lowering=False, debug=False,
                   num_devices=NCORES)

    xt_d = nc.dram_tensor("xt", (D, L), bf16, kind="ExternalInput").ap()
    wq_d = nc.dram_tensor("wq", (D, DL), bf16, kind="ExternalInput").ap()
    wk_d = nc.dram_tensor("wk", (D, DL), bf16, kind="ExternalInput").ap()
    wv_d = nc.dram_tensor("wv", (D, DL), bf16, kind="ExternalInput").ap()
    wot_d = nc.dram_tensor("wot", (DL, D), bf16, kind="ExternalInput").ap()
    cb_d = nc.dram_tensor("cb", (DL, 1), f32, kind="ExternalInput").ap()
    xres_d = nc.dram_tensor("xres", (QB, D), f32, kind="ExternalInput").ap()
    wob_d = nc.dram_tensor("wob", (1, D), f32, kind="ExternalInput").ap()
    lng_d = nc.dram_tensor("lng", (1, D), f32, kind="ExternalInput").ap()
    lnb_d = nc.dram_tensor("lnb", (1, D), f32, kind="ExternalInput").ap()
    ones_d = nc.dram_tensor("ones", (128, 128), fr, kind="ExternalInput").ap()
    vones_d = nc.dram_tensor("vones", (128, (L // 128) * HL), bf16,
                             kind="ExternalInput").ap()
    out_d = nc.dram_tensor("out", (QB, D), f32, kind="ExternalOutput").ap()

    ccin = [nc.dram_tensor(f"ccin{t}", (QB, D), f32, kind="Internal").ap()
            for t in range(NQB)]
    ccout = [nc.dram_tensor(f"ccout{t}", (QB // GROUP, D), f32,
                            kind="Internal").ap()
             for t in range(NQB)]

    with tile.TileContext(nc) as tc, contextlib.ExitStack() as ctx:
        # ---------------- persistent pools
        wp = ctx.enter_context(tc.tile_pool(name="wp", bufs=1))
        kqv = ctx.enter_context(tc.tile_pool(name="kqv", bufs=1))
        cons = ctx.enter_context(tc.tile_pool(name="cons", bufs=1))
        # two 3-bank psum slots for scores/projections + a dedicated P@v
        # accumulator pool (2 banks); the sums-broadcast rotates through psp
        psp = ctx.enter_context(tc.tile_pool(name="psp", bufs=2, space="PSUM"))
        ohp = ctx.enter_context(tc.tile_pool(name="ohp", bufs=1, space="PSUM"))
        rbp = ctx.enter_context(tc.tile_pool(name="rbp", bufs=1, space="PSUM"))

        wq_t = wp.tile([128, 8, DL], bf16)
        wk_t = wp.tile([128, 8, DL], bf16)
        wv_t = wp.tile([128, 8, DL], bf16)
        wot_t = wp.tile([128, HL, D], bf16)
        nc.sync.dma_start(out=wk_t, in_=wk_d.rearrange("(c p) o -> p c o", p=128))

        # k^T with the other head of the pair zeroed (full-square lhsT);
        # q^T keeps both heads (zero weights ignore the other head's rows)
        kt_ev = kqv.tile([128, 2, L], bf16)
        kt_od = kqv.tile([128, 2, L], bf16)
        qt = kqv.tile([128, 2, L], bf16)     # q^T (+cb)
        vaug = kqv.tile([128, L // 128, HL * 128], bf16)  # v | ones | zeros
        ohn = kqv.tile([128, HL, L], bf16)   # normalized Oh^T (rows 64+ zero)
        nc.gpsimd.memset(kt_ev[64:128, :, :], 0.0)
        nc.gpsimd.memset(kt_od[0:64, :, :], 0.0)
        nc.gpsimd.memset(vaug, 0.0)
        nc.gpsimd.memset(ohn[64:128, :, :], 0.0)

        cb_t = cons.tile([128, 2], f32)
        nc.sync.dma_start(out=cb_t, in_=cb_d.rearrange("(m p) x -> p (m x)", p=128))
        # lhsT for the sums broadcast: row 64 ones, all else zero (f32r)
        ones_t = cons.tile([128, 128], fr)
        nc.sync.dma_start(out=ones_t, in_=ones_d)
        wob_t = cons.tile([128, D], f32)
        nc.sync.dma_start(out=wob_t, in_=wob_d.partition_broadcast(128))
        lng_t = cons.tile([128, D], f32)
        nc.sync.dma_start(out=lng_t, in_=lng_d.partition_broadcast(128))
        lnb_t = cons.tile([128, D], f32)
        nc.sync.dma_start(out=lnb_t, in_=lnb_d.partition_broadcast(128))
        magic_t = cons.tile([128, 1], i32)
        nc.vector.memset(magic_t, RSQRT_MAGIC)

        nc.sync.dma_start(
            out=vaug.rearrange("p k (h x) -> p k h x", h=HL)[:, :, :, HD:HD + 1],
            in_=vones_d.rearrange("p (k h x) -> p k h x", k=L // 128, h=HL),
        )

        # ---------------- stage A: projections (needs X^T)
        with tc.tile_pool(name="xtp", bufs=1) as xtp:
            xt = xtp.tile([128, 8, L], bf16)
            for t4 in range(NQB):
                for c in range(8):
                    nc.sync.dma_start(
                        out=xt[:, c, QB * t4:QB * (t4 + 1)],
                        in_=xt_d[128 * c:128 * (c + 1), QB * t4:QB * (t4 + 1)])
            nc.sync.dma_start(out=wq_t, in_=wq_d.rearrange("(c p) o -> p c o", p=128))
            nc.sync.dma_start(out=wv_t, in_=wv_d.rearrange("(c p) o -> p c o", p=128))
            # rows 64..127 of each wot chunk are zero: they pair with the zero
            # rows of the padded ohn stationary operand (full-square matmuls
            # stream 2x faster than K<128 ones)
            nc.gpsimd.memset(wot_t[64:128, :, :], 0.0)
            nc.sync.dma_start(out=wot_t[0:64, :, :],
                              in_=wot_d.rearrange("(h p) e -> p h e", p=64))

            # k^T / q^T: out-dims on partitions, tokens free (k first:
            # its weight tile lands before wq on the DMA queues)
            for w_t, is_q in ((wk_t, False), (wq_t, True)):
                for t4 in range(NQB):
                    tsl = slice(QB * t4, QB * (t4 + 1))
                    for m in range(2):
                        ps = psp.tile([128, 3 * 512], f32, tag="ps")
                        for c in range(8):
                            nc.tensor.matmul(
                                out=ps[:, 0:512],
                                lhsT=w_t[:, c, 128 * m:128 * (m + 1)],
                                rhs=xt[:, c, tsl],
                                start=(c == 0), stop=(c == 7),
                            )
                        if is_q:
                            nc.vector.tensor_scalar_add(
                                out=qt[:, m, tsl], in0=ps[:, 0:512],
                                scalar1=cb_t[:, m:m + 1])
                        else:
                            nc.vector.tensor_copy(out=kt_ev[0:64, m, tsl],
                                                  in_=ps[0:64, 0:512])
                            nc.vector.tensor_copy(out=kt_od[64:128, m, tsl],
                                                  in_=ps[64:128, 0:512])

            # v: tokens on partitions, head dims free (augmented with ones col)
            for kc in range(L // 128):
                ps = psp.tile([128, 3 * 512], f32, tag="ps")
                for c in range(8):
                    nc.tensor.matmul(
                        out=ps[:, 0:DL],
                        lhsT=xt[:, c, 128 * kc:128 * (kc + 1)],
                        rhs=wv_t[:, c, :],
                        start=(c == 0), stop=(c == 7),
                    )
                nc.vector.tensor_copy(
                    out=vaug[:, kc, :].rearrange("p (h x) -> p h x", h=HL)[:, :, 0:HD],
                    in_=ps[:, 0:DL].rearrange("p (h x) -> p h x", h=HL),
                )
                # vaug columns 64 (ones) and 65..127 (zeros) were set up front

        # ---------------- stage B (attention) + stage C (proj/RS/LN)
        with tc.tile_pool(name="ptp", bufs=3) as ptp, \
             tc.tile_pool(name="ohsp", bufs=2) as ohsp, \
             tc.tile_pool(name="recp", bufs=2) as recp, \
             tc.tile_pool(name="zevp", bufs=2) as zevp, \
             tc.tile_pool(name="lnp", bufs=2) as lnp:

            groups = [(0, 3), (3, 3), (6, 3), (9, 3), (12, 3), (15, 1)]
            nkc = L // 128

            for qb in range(NQB):
                # ---- attention for all local heads on this query block
                for h in range(HL):
                    mi = h // 2
                    ktp = kt_ev if h % 2 == 0 else kt_od
                    qT_b = qt[:, mi, QB * qb:QB * (qb + 1)]
                    oh = ohp.tile([128, 512], f32, tag="oh")
                    for kc0, n in groups:
                        st = psp.tile([128, 3 * 512], f32, tag="ps")
                        for i in range(n):
                            kc = kc0 + i
                            nc.tensor.matmul(
                                out=st[:, 512 * i:512 * (i + 1)],
                                lhsT=ktp[:, mi, 128 * kc:128 * (kc + 1)],
                                rhs=qT_b,
                                start=True, stop=True,
                            )
                        pt = ptp.tile([128, 3 * 512], bf16, tag="pt")
                        nc.scalar.activation(out=pt[:, :512 * n],
                                             in_=st[:, :512 * n], func=Exp)
                        for i in range(n):
                            kc = kc0 + i
                            nc.tensor.matmul(
                                out=oh,
                                lhsT=vaug[:, kc, 128 * h:128 * (h + 1)],
                                rhs=pt[:, 512 * i:512 * (i + 1)],
                                start=(kc == 0), stop=(kc == nkc - 1),
                            )
                    # evacuate Oh + sums, broadcast sums, reciprocal, scale
                    ohs = ohsp.tile([65, 512], fr, tag="ohs")
                    with nc.allow_low_precision(reason="f32r rounding of Oh"):
                        nc.vector.tensor_copy(out=ohs, in_=oh[0:65, :])
                    rb = rbp.tile([128, 512], f32, tag="rb")
                    nc.tensor.matmul(out=rb, lhsT=ones_t[0:65, :],
                                     rhs=ohs, start=True, stop=True)
                    rec = recp.tile([64, 512], f32, tag="rec")
                    nc.vector.reciprocal_approx_fast(out=rec, in_=rb[0:64, :])
                    nc.vector.tensor_mul(
                        out=ohn[0:64, h, QB * qb:QB * (qb + 1)],
                        in0=ohs[0:64, :], in1=rec)

                # ---- output projection partial for this token block
                for tcl in range(QB // 128):
                    t0 = QB * qb + 128 * tcl
                    zev = zevp.tile([128, D], f32)
                    for ec in range(2):
                        zp = psp.tile([128, 3 * 512], f32, tag="ps")
                        for h in range(HL):
                            nc.tensor.matmul(
                                out=zp[:, 0:512],
                                lhsT=ohn[:, h, t0:t0 + 128],
                                rhs=wot_t[:, h, 512 * ec:512 * (ec + 1)],
                                start=(h == 0), stop=(h == HL - 1),
                            )
                        nc.vector.tensor_copy(out=zev[:, 512 * ec:512 * (ec + 1)],
                                              in_=zp[:, 0:512])
                    nc.sync.dma_start(
                        out=ccin[qb][128 * tcl:128 * (tcl + 1), :], in_=zev)

                # ---- combine partials across the batch group
                nc.gpsimd.collective_compute(
                    "ReduceScatter", Alu.add,
                    ins=[ccin[qb][:]], outs=[ccout[qb][:]],
                    replica_groups=RG,
                )

            # ---- residual + bias + LayerNorm, deferred so the in-order DVE
            # stream never blocks attention work behind a ReduceScatter wait
            for qb in range(NQB):
              with tc.tile_wait_until(0.20 + 0.04 * qb):
                zt = lnp.tile([128, D], f32, tag="zt")
                nc.sync.dma_start(out=zt, in_=ccout[qb])
                xr = lnp.tile([128, D], f32, tag="xr")
                nc.sync.dma_start(out=xr, in_=xres_d[128 * qb:128 * (qb + 1), :])
                nc.vector.tensor_add(out=zt, in0=zt, in1=xr)
                nc.vector.tensor_add(out=zt, in0=zt, in1=wob_t)

                stats = lnp.tile([128, 2, 6], f32, tag="stats")
                for sg in range(2):
                    nc.vector.bn_stats(out=stats[:, sg, :],
                                       in_=zt[:, 512 * sg:512 * (sg + 1)])
                mv = lnp.tile([128, 2], f32, tag="mv")
                nc.vector.bn_aggr(out=mv, in_=stats)

                # rstd = rsqrt(var + eps), DVE-only (avoids ACT table thrash)
                ve = lnp.tile([128, 1], f32, tag="ve")
                nc.vector.tensor_scalar_add(out=ve, in0=mv[:, 1:2], scalar1=LN_EPS)
                y = lnp.tile([128, 1], f32, tag="y")
                nc.vector.tensor_scalar(
                    out=y.bitcast(i32), in0=ve.bitcast(i32), scalar1=1,
                    scalar2=None, op0=Alu.logical_shift_right)
                nc.vector.tensor_sub(out=y.bitcast(i32), in0=magic_t,
                                     in1=y.bitcast(i32))
                tnw = lnp.tile([128, 1], f32, tag="tnw")
                for _ in range(3):
                    nc.vector.tensor_mul(out=tnw, in0=ve, in1=y)
                    nc.vector.tensor_mul(out=tnw, in0=tnw, in1=y)
                    nc.vector.tensor_scalar(out=tnw, in0=tnw, scalar1=-0.5,
                                            scalar2=1.5, op0=Alu.mult, op1=Alu.add)
                    nc.vector.tensor_mul(out=y, in0=y, in1=tnw)

                nc.vector.tensor_scalar(out=zt, in0=zt, scalar1=mv[:, 0:1],
                                        scalar2=y, op0=Alu.subtract, op1=Alu.mult)
                nc.vector.tensor_mul(out=zt, in0=zt, in1=lng_t)
                nc.vector.tensor_add(out=zt, in0=zt, in1=lnb_t)
                nc.sync.dma_start(out=out_d[128 * qb:128 * (qb + 1), :], in_=zt)


    nc.compile()
    return nc


def _get_program():
    global _PROGRAM
    if _PROGRAM is None:
        _PROGRAM = _build_program()
    return _PROGRAM


def kernel(X, Y, Wq, Wk, Wv, cb, Wo_w, Wo_b, ln_g, ln_b):
    import ml_dtypes
    from concourse import bass_utils

    prog = _get_program()
    bf = ml_dtypes.bfloat16

    X = np.asarray(X, dtype=np.float32)
    Wq = np.asarray(Wq, dtype=np.float32)
    Wk = np.asarray(Wk, dtype=np.float32)
    Wv = np.asarray(Wv, dtype=np.float32)
    cb = np.asarray(cb, dtype=np.float32)
    Wo_w = np.asarray(Wo_w, dtype=np.float32)
    Wo_b = np.asarray(Wo_b, dtype=np.float32)
    ln_g = np.asarray(ln_g, dtype=np.float32)
    ln_b = np.asarray(ln_b, dtype=np.float32)

    WoT = np.ascontiguousarray(Wo_w.T)
    ones_arr = np.zeros((128, 128), np.float32)
    ones_arr[64, :] = 1.0
    in_maps = []
    for c in range(NCORES):
        b, hp, r = c // GROUP, c % GROUP, c % GROUP
        Xb = X[b]
        rows = np.concatenate(
            [np.arange(QB * t + 128 * r, QB * t + 128 * r + 128)
             for t in range(NQB)])
        csl = slice(DL * hp, DL * (hp + 1))
        in_maps.append({
            "xt": np.ascontiguousarray(Xb.T).astype(bf),
            "xres": np.ascontiguousarray(Xb[rows]),
            "wq": np.ascontiguousarray(Wq[:, csl]).astype(bf),
            "wk": np.ascontiguousarray(Wk[:, csl]).astype(bf),
            "wv": np.ascontiguousarray(Wv[:, csl]).astype(bf),
            "wot": np.ascontiguousarray(WoT[csl, :]).astype(bf),
            "cb": np.ascontiguousarray(cb[csl].reshape(DL, 1)),
            "wob": np.ascontiguousarray(Wo_b.reshape(1, D)),
            "lng": np.ascontiguousarray(ln_g.reshape(1, D)),
            "lnb": np.ascontiguousarray(ln_b.reshape(1, D)),
            "ones": ones_arr,
            "vones": np.ones((128, (L // 128) * HL), bf),
        })

    res = bass_utils.run_bass_kernel_spmd(prog, in_maps, core_ids=list(range(NCORES)))
    global LAST_RESULT
    LAST_RESULT = res

    out = np.empty((B, L, D), np.float32)
    for cid in range(NCORES):
        b, r = cid // GROUP, cid % GROUP
        o = res.results[cid]["out"]
        for t in range(NQB):
            out[b, QB * t + 128 * r:QB * t + 128 * r + 128] = o[128 * t:128 * (t + 1)]
    return out


if __name__ == "__main__":
    rng = np.random.default_rng(0)
    ins = {
        "X": rng.standard_normal((B, L, D)).astype(np.float32),
        "Y": rng.standard_normal((B, L, D)).astype(np.float32),
        "Wq": (rng.uniform(-1, 1, (D, D)) / 32).astype(np.float32),
        "Wk": (rng.uniform(-1, 1, (D, D)) / 32).astype(np.float32),
        "Wv": (rng.uniform(-1, 1, (D, D)) / 32).astype(np.float32),
        "cb": np.zeros(D, np.float32),
        "Wo_w": (rng.uniform(-1, 1, (D, D)) / 32).astype(np.float32),
        "Wo_b": (rng.uniform(-1, 1, D) / 32).astype(np.float32),
        "ln_g": np.ones(D, np.float32),
        "ln_b": np.zeros(D, np.float32),
    }
    out = kernel(**ins)
    print("out", out.shape, out.dtype, float(np.abs(out).max()))
    print("exec_time_ns:", LAST_RESULT.exec_time_ns)


# revision 36
# speedup vs baseline: 1.1782x; 1.0124x over previous
"""Trainium2 Bass kernel for nn_Attention_90220083019846.

Multi-head attention block: q/k/v = X@W{q,k,v}, scores = q@k^T + cb@k^T
(content bias folded into q), softmax, O = P@v, Z = X + O@Wo^T + b, LayerNorm.

Sharding over 8 NeuronCores: data-parallel over batch (2 groups of 4 cores) x
tensor-parallel over heads (4 heads per core). Output projection partial sums
are combined with a chunked ReduceScatter within each batch group; residual +
LayerNorm run on the scattered shards.

Dataflow is fully "transposed": the host passes X^T, so every matmul contracts
over the partition axis with no on-device transposes. Matmuls run in bf16
(f32 PSUM accumulation); softmax sums come free from the P@v matmul via a
fused ones-column in v (M=65). PSUM is organized as two 4-bank slots: score
tiles, their exp-consumers, the P@v partial accumulators and all projection
accumulations rotate through the same two slots, with cross-group O
accumulation done in SBUF by the vector engine.
"""

import contextlib
import ctypes
import sys
import types

sys.path.insert(0, "/opt/trn_rl_repo")

import numpy as np

# ---------------------------------------------------------------- profile hook
# The agent image's antenv lacks axon_hooks; provide it so that
# run_bass_kernel_spmd(trace=True) / BASS_TRACE=1 can capture NTFF profiles.
def _install_profile_hook():
    if "antenv.axon_hooks" in sys.modules:
        return
    try:
        import antenv
    except ImportError:
        return
    mod = types.ModuleType("antenv.axon_hooks")
    mod._hook = None
    mod.set_axon_ntff_profile_hook = lambda h: setattr(mod, "_hook", h)
    mod.get_axon_ntff_profile_hook = lambda: mod._hook
    sys.modules["antenv.axon_hooks"] = mod
    antenv.axon_hooks = mod
    try:
        lib = ctypes.CDLL("/opt/axon/libaxon_pjrt.so")
        if not hasattr(lib, "axon_start_nrt_profile"):
            return
        lib.axon_start_nrt_profile.argtypes = [
            ctypes.POINTER(ctypes.c_int64),
            ctypes.c_size_t,
        ]
        lib.axon_start_nrt_profile.restype = ctypes.c_int64
        lib.axon_stop_nrt_profile.argtypes = [ctypes.c_char_p]
        lib.axon_stop_nrt_profile.restype = ctypes.c_int64

        @contextlib.contextmanager
        def _hook(output_dir, device_ids):
            import jax

            jax.devices()
            if device_ids:
                ids = (ctypes.c_int64 * len(device_ids))(*device_ids)
                rc = lib.axon_start_nrt_profile(ids, len(device_ids))
            else:
                rc = lib.axon_start_nrt_profile(None, 0)
            if rc != 0:
                raise RuntimeError(f"axon_start_nrt_profile rc={rc}")
            try:
                yield
            finally:
                n = lib.axon_stop_nrt_profile(str(output_dir).encode())
                print(f"profile: {n} file(s) written to {output_dir}", file=sys.stderr)

        mod.set_axon_ntff_profile_hook(_hook)
    except OSError:
        pass


_install_profile_hook()

# ------------------------------------------------------------------- constants
B, L, D, H, HD = 2, 2048, 1024, 16, 64
NCORES = 8
GROUP = 4            # cores per batch group (tensor-parallel over heads)
HL = H // GROUP      # local heads per core
DL = HL * HD         # local head dims per core
QB = 512             # query block (tokens per pipeline chunk)
NQB = L // QB
RG = [[0, 1, 2, 3], [4, 5, 6, 7]]
LN_EPS = 1e-5
RSQRT_MAGIC = 0x5F3759DF

_PROGRAM = None
LAST_RESULT = None


def _build_program():
    import concourse.tile as tile
    from concourse import bacc, mybir

    fr = mybir.dt.float32r
    f32 = mybir.dt.float32
    bf16 = mybir.dt.bfloat16
    i32 = mybir.dt.int32
    Exp = mybir.ActivationFunctionType.Exp
    Alu = mybir.AluOpType

    nc = bacc.Bacc("TRN2", target_bir_lowering=False, debug=False,
                   num_devices=NCORES)

    xt_d = nc.dram_tensor("xt", (D, L), bf16, kind="ExternalInput").ap()
    wq_d = nc.dram_tensor("wq", (D, DL), bf16, kind="ExternalInput").ap()
    wk_d = nc.dram_tensor("wk", (D, DL), bf16, kind="ExternalInput").ap()
    wv_d = nc.dram_tensor("wv", (D, DL), bf16, kind="ExternalInput").ap()
    wot_d = nc.dram_tensor("wot", (DL, D), bf16, kind="ExternalInput").ap()
    cb_d = nc.dram_tensor("cb", (DL, 1), f32, kind="ExternalInput").ap()
    xres_d = nc.dram_tensor("xres", (QB, D), f32, kind="ExternalInput").ap()
    wob_d = nc.dram_tensor("wob", (1, D), f32, kind="ExternalInput").ap()
    lng_d = nc.dram_tensor("lng", (1, D), f32, kind="ExternalInput").ap()
    lnb_d = nc.dram_tensor("lnb", (1, D), f32, kind="ExternalInput").ap()
    ones_d = nc.dram_tensor("ones", (128, 128), fr, kind="ExternalInput").ap()
    vones_d = nc.dram_tensor("vones", (128, (L // 128) * HL), bf16,
                             kind="ExternalInput").ap()
    out_d = nc.dram_tensor("out", (QB, D), f32, kind="ExternalOutput").ap()

    ccin = [nc.dram_tensor(f"ccin{t}", (QB, D), f32, kind="Internal").ap()
            for t in range(NQB - 1)]
    ccout = [nc.dram_tensor(f"ccout{t}", (QB // GROUP, D), f32,
                            kind="Internal").ap()
             for t in range(NQB - 1)]
    ccin_l = [nc.dram_tensor(f"ccinl{t}", (QB // 2, D), f32, kind="Internal").ap()
              for t in range(2)]
    ccout_l = [nc.dram_tensor(f"ccoutl{t}", (QB // 2 // GROUP, D), f32,
                              kind="Internal").ap()
               for t in range(2)]

    with tile.TileContext(nc) as tc, contextlib.ExitStack() as ctx:
        # ---------------- persistent pools
        wp = ctx.enter_context(tc.tile_pool(name="wp", bufs=1))
        kqv = ctx.enter_context(tc.tile_pool(name="kqv", bufs=1))
        cons = ctx.enter_context(tc.tile_pool(name="cons", bufs=1))
        # two 3-bank psum slots for scores/projections + a dedicated P@v
        # accumulator pool (2 banks); the sums-broadcast rotates through psp
        psp = ctx.enter_context(tc.tile_pool(name="psp", bufs=2, space="PSUM"))
        ohp = ctx.enter_context(tc.tile_pool(name="ohp", bufs=1, space="PSUM"))
        rbp = ctx.enter_context(tc.tile_pool(name="rbp", bufs=1, space="PSUM"))

        wq_t = wp.tile([128, 8, DL], bf16)
        wk_t = wp.tile([128, 8, DL], bf16)
        wv_t = wp.tile([128, 8, DL], bf16)
        wot_t = wp.tile([128, HL, D], bf16)
        nc.sync.dma_start(out=wk_t, in_=wk_d.rearrange("(c p) o -> p c o", p=128))

        # k^T with the other head of the pair zeroed (full-square lhsT);
        # q^T keeps both heads (zero weights ignore the other head's rows)
        kt_ev = kqv.tile([128, 2, L], bf16)
        kt_od = kqv.tile([128, 2, L], bf16)
        qt = kqv.tile([128, 2, L], bf16)     # q^T (+cb)
        vaug = kqv.tile([128, L // 128, HL * 128], bf16)  # v | ones | zeros
        ohn = kqv.tile([128, HL, L], bf16)   # normalized Oh^T (rows 64+ zero)
        nc.vector.memset(vaug, 0.0)
        nc.gpsimd.memset(ohn[64:128, :, :], 0.0)

        cb_t = cons.tile([128, 2], f32)
        nc.sync.dma_start(out=cb_t, in_=cb_d.rearrange("(m p) x -> p (m x)", p=128))
        # lhsT for the sums broadcast: row 64 ones, all else zero (f32r)
        ones_t = cons.tile([128, 128], fr)
        nc.sync.dma_start(out=ones_t, in_=ones_d)
        wob_t = cons.tile([128, D], f32)
        nc.sync.dma_start(out=wob_t, in_=wob_d.partition_broadcast(128))
        lng_t = cons.tile([128, D], f32)
        nc.sync.dma_start(out=lng_t, in_=lng_d.partition_broadcast(128))
        lnb_t = cons.tile([128, D], f32)
        nc.sync.dma_start(out=lnb_t, in_=lnb_d.partition_broadcast(128))
        magic_t = cons.tile([128, 1], i32)
        nc.vector.memset(magic_t, RSQRT_MAGIC)
        # row masks: keep one head of a pair, zero the other (f32 scalars)
        mask_lo = cons.tile([128, 1], f32)
        mask_hi = cons.tile([128, 1], f32)
        nc.vector.memset(mask_lo, 0.0)
        nc.vector.memset(mask_lo[0:64, :], 1.0)
        nc.vector.memset(mask_hi, 0.0)
        nc.vector.memset(mask_hi[64:128, :], 1.0)

        nc.sync.dma_start(
            out=vaug.rearrange("p k (h x) -> p k h x", h=HL)[:, :, :, HD:HD + 1],
            in_=vones_d.rearrange("p (k h x) -> p k h x", k=L // 128, h=HL),
        )

        # ---------------- stage A: projections (needs X^T)
        with tc.tile_pool(name="xtp", bufs=1) as xtp:
            xt = xtp.tile([128, 8, L], bf16)
            for t4 in range(NQB):
                for c in range(8):
                    nc.sync.dma_start(
                        out=xt[:, c, QB * t4:QB * (t4 + 1)],
                        in_=xt_d[128 * c:128 * (c + 1), QB * t4:QB * (t4 + 1)])
            nc.sync.dma_start(out=wq_t, in_=wq_d.rearrange("(c p) o -> p c o", p=128))
            nc.sync.dma_start(out=wv_t, in_=wv_d.rearrange("(c p) o -> p c o", p=128))
            # rows 64..127 of each wot chunk are zero: they pair with the zero
            # rows of the padded ohn stationary operand (full-square matmuls
            # stream 2x faster than K<128 ones)
            nc.gpsimd.memset(wot_t[64:128, :, :], 0.0)
            nc.sync.dma_start(out=wot_t[0:64, :, :],
                              in_=wot_d.rearrange("(h p) e -> p h e", p=64))

            # k^T / q^T: out-dims on partitions, tokens free (k first:
            # its weight tile lands before wq on the DMA queues)
            for w_t, is_q in ((wk_t, False), (wq_t, True)):
                for t4 in range(NQB):
                    tsl = slice(QB * t4, QB * (t4 + 1))
                    for m in range(2):
                        ps = psp.tile([128, 3 * 512], f32, tag="ps")
                        for c in range(8):
                            nc.tensor.matmul(
                                out=ps[:, 0:512],
                                lhsT=w_t[:, c, 128 * m:128 * (m + 1)],
                                rhs=xt[:, c, tsl],
                                start=(c == 0), stop=(c == 7),
                            )
                        if is_q:
                            nc.vector.tensor_scalar_add(
                                out=qt[:, m, tsl], in0=ps[:, 0:512],
                                scalar1=cb_t[:, m:m + 1])
                        else:
                            nc.vector.tensor_scalar_mul(
                                out=kt_ev[:, m, tsl], in0=ps[:, 0:512],
                                scalar1=mask_lo)
                            nc.vector.tensor_scalar_mul(
                                out=kt_od[:, m, tsl], in0=ps[:, 0:512],
                                scalar1=mask_hi)

            # v: tokens on partitions, head dims free (augmented with ones col)
            for kc in range(L // 128):
                ps = psp.tile([128, 3 * 512], f32, tag="ps")
                for c in range(8):
                    nc.tensor.matmul(
                        out=ps[:, 0:DL],
                        lhsT=xt[:, c, 128 * kc:128 * (kc + 1)],
                        rhs=wv_t[:, c, :],
                        start=(c == 0), stop=(c == 7),
                    )
                nc.vector.tensor_copy(
                    out=vaug[:, kc, :].rearrange("p (h x) -> p h x", h=HL)[:, :, 0:HD],
                    in_=ps[:, 0:DL].rearrange("p (h x) -> p h x", h=HL),
                )
                # vaug columns 64 (ones) and 65..127 (zeros) were set up front

        # ---------------- stage B (attention) + stage C (proj/RS/LN)
        with tc.tile_pool(name="ptp", bufs=3) as ptp, \
             tc.tile_pool(name="ohsp", bufs=2) as ohsp, \
             tc.tile_pool(name="recp", bufs=2) as recp, \
             tc.tile_pool(name="zevp", bufs=2) as zevp, \
             tc.tile_pool(name="lnp", bufs=2) as lnp:

            groups = [(0, 3), (3, 3), (6, 3), (9, 3), (12, 3), (15, 1)]
            nkc = L // 128

            for qb in range(NQB):
                # ---- attention for all local heads on this query block
                for h in range(HL):
                    mi = h // 2
                    ktp = kt_ev if h % 2 == 0 else kt_od
                    qT_b = qt[:, mi, QB * qb:QB * (qb + 1)]
                    oh = ohp.tile([128, 512], f32, tag="oh")
                    for kc0, n in groups:
                        st = psp.tile([128, 3 * 512], f32, tag="ps")
                        for i in range(n):
                            kc = kc0 + i
                            nc.tensor.matmul(
                                out=st[:, 512 * i:512 * (i + 1)],
                                lhsT=ktp[:, mi, 128 * kc:128 * (kc + 1)],
                                rhs=qT_b,
                                start=True, stop=True,
                            )
                        pt = ptp.tile([128, 3 * 512], bf16, tag="pt")
                        nc.scalar.activation(out=pt[:, :512 * n],
                                             in_=st[:, :512 * n], func=Exp)
                        for i in range(n):
                            kc = kc0 + i
                            nc.tensor.matmul(
                                out=oh,
                                lhsT=vaug[:, kc, 128 * h:128 * (h + 1)],
                                rhs=pt[:, 512 * i:512 * (i + 1)],
                                start=(kc == 0), stop=(kc == nkc - 1),
                            )
                    # evacuate Oh + sums, broadcast sums, reciprocal, scale
                    ohs = ohsp.tile([65, 512], fr, tag="ohs")
                    with nc.allow_low_precision(reason="f32r rounding of Oh"):
                        nc.vector.tensor_copy(out=ohs, in_=oh[0:65, :])
                    rb = rbp.tile([128, 512], f32, tag="rb")
                    nc.tensor.matmul(out=rb, lhsT=ones_t[0:65, :],
                                     rhs=ohs, start=True, stop=True)
                    rec = recp.tile([64, 512], f32, tag="rec")
                    nc.vector.reciprocal_approx_fast(out=rec, in_=rb[0:64, :])
                    nc.vector.tensor_mul(
                        out=ohn[0:64, h, QB * qb:QB * (qb + 1)],
                        in0=ohs[0:64, :], in1=rec)

                # ---- output projection partial for this token block
                for tcl in range(QB // 128):
                    t0 = QB * qb + 128 * tcl
                    zev = zevp.tile([128, D], f32)
                    for ec in range(2):
                        zp = psp.tile([128, 3 * 512], f32, tag="ps")
                        for h in range(HL):
                            nc.tensor.matmul(
                                out=zp[:, 0:512],
                                lhsT=ohn[:, h, t0:t0 + 128],
                                rhs=wot_t[:, h, 512 * ec:512 * (ec + 1)],
                                start=(h == 0), stop=(h == HL - 1),
                            )
                        nc.vector.tensor_copy(out=zev[:, 512 * ec:512 * (ec + 1)],
                                              in_=zp[:, 0:512])
                    if qb < NQB - 1:
                        nc.sync.dma_start(
                            out=ccin[qb][128 * tcl:128 * (tcl + 1), :], in_=zev)
                    else:
                        nc.sync.dma_start(
                            out=ccin_l[tcl // 2][128 * (tcl % 2):128 * (tcl % 2 + 1), :],
                            in_=zev)
                        if tcl % 2 == 1:
                            nc.gpsimd.collective_compute(
                                "ReduceScatter", Alu.add,
                                ins=[ccin_l[tcl // 2][:]],
                                outs=[ccout_l[tcl // 2][:]],
                                replica_groups=RG,
                            )

                # ---- combine partials across the batch group (full blocks)
                if qb < NQB - 1:
                    nc.gpsimd.collective_compute(
                        "ReduceScatter", Alu.add,
                        ins=[ccin[qb][:]], outs=[ccout[qb][:]],
                        replica_groups=RG,
                    )

            # ---- residual + bias + LayerNorm, deferred so the in-order DVE
            # stream never blocks attention work behind a ReduceScatter wait
            for qb in range(NQB):
              with tc.tile_wait_until(0.23 + 0.045 * qb):
                zt = lnp.tile([128, D], f32, tag="zt")
                if qb < NQB - 1:
                    nc.sync.dma_start(out=zt, in_=ccout[qb])
                else:
                    nc.sync.dma_start(out=zt[0:64, :], in_=ccout_l[0])
                    nc.sync.dma_start(out=zt[64:128, :], in_=ccout_l[1])
                xr = lnp.tile([128, D], f32, tag="xr")
                nc.sync.dma_start(out=xr, in_=xres_d[128 * qb:128 * (qb + 1), :])
                nc.vector.tensor_add(out=zt, in0=zt, in1=xr)
                nc.vector.tensor_add(out=zt, in0=zt, in1=wob_t)

                stats = lnp.tile([128, 2, 6], f32, tag="stats")
                for sg in range(2):
                    nc.vector.bn_stats(out=stats[:, sg, :],
                                       in_=zt[:, 512 * sg:512 * (sg + 1)])
                mv = lnp.tile([128, 2], f32, tag="mv")
                nc.vector.bn_aggr(out=mv, in_=stats)

                # rstd = rsqrt(var + eps), DVE-only (avoids ACT table thrash)
                ve = lnp.tile([128, 1], f32, tag="ve")
                nc.vector.tensor_scalar_add(out=ve, in0=mv[:, 1:2], scalar1=LN_EPS)
                y = lnp.tile([128, 1], f32, tag="y")
                nc.vector.tensor_scalar(
                    out=y.bitcast(i32), in0=ve.bitcast(i32), scalar1=1,
                    scalar2=None, op0=Alu.logical_shift_right)
                nc.vector.tensor_sub(out=y.bitcast(i32), in0=magic_t,
                                     in1=y.bitcast(i32))
                tnw = lnp.tile([128, 1], f32, tag="tnw")
                for _ in range(3):
                    nc.vector.tensor_mul(out=tnw, in0=ve, in1=y)
                    nc.vector.tensor_mul(out=tnw, in0=tnw, in1=y)
                    nc.vector.tensor_scalar(out=tnw, in0=tnw, scalar1=-0.5,
                                            scalar2=1.5, op0=Alu.mult, op1=Alu.add)
                    nc.vector.tensor_mul(out=y, in0=y, in1=tnw)

                nc.vector.tensor_scalar(out=zt, in0=zt, scalar1=mv[:, 0:1],
                                        scalar2=y, op0=Alu.subtract, op1=Alu.mult)
                nc.vector.tensor_mul(out=zt, in0=zt, in1=lng_t)
                nc.vector.tensor_add(out=zt, in0=zt, in1=lnb_t)
                nc.sync.dma_start(out=out_d[128 * qb:128 * (qb + 1), :], in_=zt)


    nc.compile()
    return nc


def _get_program():
    global _PROGRAM
    if _PROGRAM is None:
        _PROGRAM = _build_program()
    return _PROGRAM


def kernel(X, Y, Wq, Wk, Wv, cb, Wo_w, Wo_b, ln_g, ln_b):
    import ml_dtypes
    from concourse import bass_utils

    prog = _get_program()
    bf = ml_dtypes.bfloat16

    X = np.asarray(X, dtype=np.float32)
    Wq = np.asarray(Wq, dtype=np.float32)
    Wk = np.asarray(Wk, dtype=np.float32)
    Wv = np.asarray(Wv, dtype=np.float32)
    cb = np.asarray(cb, dtype=np.float32)
    Wo_w = np.asarray(Wo_w, dtype=np.float32)
    Wo_b = np.asarray(Wo_b, dtype=np.float32)
    ln_g = np.asarray(ln_g, dtype=np.float32)
    ln_b = np.asarray(ln_b, dtype=np.float32)

    WoT = np.ascontiguousarray(Wo_w.T)
    ones_arr = np.zeros((128, 128), np.float32)
    ones_arr[64, :] = 1.0
    in_maps = []
    for c in range(NCORES):
        b, hp, r = c // GROUP, c % GROUP, c % GROUP
        Xb = X[b]
        rows = np.concatenate(
            [np.arange(QB * t + 128 * r, QB * t + 128 * r + 128)
             for t in range(NQB - 1)]
            + [np.arange(QB * (NQB - 1) + 256 * hh + 64 * r,
                         QB * (NQB - 1) + 256 * hh + 64 * r + 64)
               for hh in range(2)])
        csl = slice(DL * hp, DL * (hp + 1))
        in_maps.append({
            "xt": np.ascontiguousarray(Xb.T).astype(bf),
            "xres": np.ascontiguousarray(Xb[rows]),
            "wq": np.ascontiguousarray(Wq[:, csl]).astype(bf),
            "wk": np.ascontiguousarray(Wk[:, csl]).astype(bf),
            "wv": np.ascontiguousarray(Wv[:, csl]).astype(bf),
            "wot": np.ascontiguousarray(WoT[csl, :]).astype(bf),
            "cb": np.ascontiguousarray(cb[csl].reshape(DL, 1)),
            "wob": np.ascontiguousarray(Wo_b.reshape(1, D)),
            "lng": np.ascontiguousarray(ln_g.reshape(1, D)),
            "lnb": np.ascontiguousarray(ln_b.reshape(1, D)),
            "ones": ones_arr,
            "vones": np.ones((128, (L // 128) * HL), bf),
        })

    res = bass_utils.run_bass_kernel_spmd(prog, in_maps, core_ids=list(range(NCORES)))
    global LAST_RESULT
    LAST_RESULT = res

    out = np.empty((B, L, D), np.float32)
    for cid in range(NCORES):
        b, r = cid // GROUP, cid % GROUP
        o = res.results[cid]["out"]
        for t in range(NQB - 1):
            out[b, QB * t + 128 * r:QB * t + 128 * r + 128] = o[128 * t:128 * (t + 1)]
        for hh in range(2):
            g0 = QB * (NQB - 1) + 256 * hh + 64 * r
            out[b, g0:g0 + 64] = o[128 * (NQB - 1) + 64 * hh:
                                   128 * (NQB - 1) + 64 * (hh + 1)]
    return out


if __name__ == "__main__":
    rng = np.random.default_rng(0)
    ins = {
        "X": rng.standard_normal((B, L, D)).astype(np.float32),
        "Y": rng.standard_normal((B, L, D)).astype(np.float32),
        "Wq": (rng.uniform(-1, 1, (D, D)) / 32).astype(np.float32),
        "Wk": (rng.uniform(-1, 1, (D, D)) / 32).astype(np.float32),
        "Wv": (rng.uniform(-1, 1, (D, D)) / 32).astype(np.float32),
        "cb": np.zeros(D, np.float32),
        "Wo_w": (rng.uniform(-1, 1, (D, D)) / 32).astype(np.float32),
        "Wo_b": (rng.uniform(-1, 1, D) / 32).astype(np.float32),
        "ln_g": np.ones(D, np.float32),
        "ln_b": np.zeros(D, np.float32),
    }
    out = kernel(**ins)
    print("out", out.shape, out.dtype, float(np.abs(out).max()))
    print("exec_time_ns:", LAST_RESULT.exec_time_ns)
